# revision 1
# baseline (speedup 1.0000x reference)
"""NetVLAD (vq_codebook) Trainium2 Bass kernel, 8-way spatially sharded.

Math (verified vs reference to ~2e-7 rel):
  xn = x / ||x||_C per location; logits = conv_w @ xn; soft = softmax_K
  fold(unfold(soft) * top2keep) == soft * cnt, where cnt = 3x3 box-sum of the
  per-cluster top-2 indicator (border discrepancies are killed by the
  (min-dist-to-border)^4 mask). vlad = sa2 @ xn.T - rowsum(sa2) * centroids,
  then intra + global L2 norm.

Sharding: H=192 rows split 8 ways (24 rows/core + 1 halo row each side).
conv/softmax/top2/box-sum local per core; [K,C+1] partial VLAD sums
all-reduced across the 8 cores; final normalization redundantly on each core.
"""
import os
import sys

sys.path.insert(0, "/opt/trn_rl_repo")
os.environ.setdefault("MYCRO_LOCAL_CACHE", "1")

import numpy as np

C, H, W, K = 512, 192, 192, 64
M = 8                      # cores
RPC = H // M               # 24 rows per core
Ls = (RPC + 2) * W         # 4992 slab locations (incl. 1 halo row each side)
NT = Ls // 128             # 39 l-tiles
CT = C // 128              # 4 c-tiles
G = 257                    # odd guard -> v-pass offsets even (bf16 2x mode)
KBW = G + Ls + G           # 5506 keep-buffer width
XW = 8                     # xlc DMA batching (tiles per DMA)

TRACE = False              # set by test.py for profiling runs
_CACHE = {}


def _build_nc():
    import concourse.bass as bass
    import concourse.bass_isa as bass_isa
    import concourse.tile as tile
    from concourse import mybir

    f32 = mybir.dt.float32
    bf16 = mybir.dt.bfloat16
    AF = mybir.ActivationFunctionType
    OP = mybir.AluOpType
    AX = mybir.AxisListType

    nc = bass.Bass()
    xcl = nc.dram_tensor("xcl", [C, Ls], f32, kind="ExternalInput")
    xlcn = nc.dram_tensor("xlcn", [Ls, C + 1], f32, kind="ExternalInput")
    cwt = nc.dram_tensor("cwt", [C, K], f32, kind="ExternalInput")
    cent = nc.dram_tensor("cent", [K, C], f32, kind="ExternalInput")
    sc0 = nc.dram_tensor("sc0", [128, NT], f32, kind="ExternalInput")
    invn = nc.dram_tensor("invn", [128, NT], f32, kind="ExternalInput")
    identb = nc.dram_tensor("identb", [128, 128], bf16, kind="ExternalInput")
    identf = nc.dram_tensor("identf", [128, 128], f32, kind="ExternalInput")
    ones = nc.dram_tensor("ones", [128, 128], f32, kind="ExternalInput")
    y = nc.dram_tensor("y", [K, C + 1], f32, kind="ExternalOutput")

    with tile.TileContext(nc) as tc:
        with tc.tile_pool(name="big", bufs=1) as big:
            # persistent SBUF tensors
            expb = big.tile([128, NT * K], f32, tag="expb")
            tmpb = big.tile([128, NT * K], f32, tag="tmpb")   # also reused as w2
            keeplk = big.tile([128, NT * K], bf16, tag="keeplk")
            kb = big.tile([K, KBW], bf16, tag="kb")
            h3s = big.tile([K, KBW], bf16, tag="h3s")
            cntb = big.tile([K, Ls], bf16, tag="cntb")
            cwt_sb = big.tile([128, CT * K], f32, tag="cwt")
            cent_sb = big.tile([K, C], f32, tag="cent")
            id_sb = big.tile([128, 128], bf16, tag="ident")
            idf_sb = big.tile([128, 128], f32, tag="identf")
            logkl = big.tile([K, Ls], f32, tag="logkl")
            ones_sb = big.tile([128, 128], f32, tag="ones")
            sc_sb = big.tile([128, NT], f32, tag="sc0")
            invn_sb = big.tile([128, NT], f32, tag="invn")
            sume = big.tile([128, NT], f32, tag="sume")
            m1b = big.tile([128, NT], f32, tag="m1b")
            m2b = big.tile([128, NT], f32, tag="m2b")
            isum = big.tile([128, NT], f32, tag="isum")
            scc = big.tile([128, NT], f32, tag="scc")
            vl_sb = big.tile([K, C + 1], f32, tag="vl")
            scr = big.tile([128, 4], f32, tag="scr")

            # input DMAs
            nc.sync.dma_start(id_sb[:], identb[:])
            nc.sync.dma_start(idf_sb[:], identf[:])
            nc.sync.dma_start(ones_sb[:], ones[:])
            nc.sync.dma_start(sc_sb[:], sc0[:])
            nc.sync.dma_start(invn_sb[:], invn[:])
            nc.sync.dma_start(cent_sb[:], cent[:])
            nc.sync.dma_start(
                cwt_sb[:].rearrange("p (t k) -> p t k", k=K),
                cwt[:].rearrange("(t p) k -> p t k", p=128),
            )
            # zero the keep-buffer guards
            nc.vector.memset(kb[:, 0:G], 0.0)
            nc.vector.memset(kb[:, G + Ls:KBW], 0.0)
            # single-wait "touch" ops: each absorbs one DMA completion so no
            # downstream compute instruction needs two sync waits (codegen
            # allows one wait per compute-engine instruction)
            nc.scalar.copy(scr[:, 0:1], invn_sb[:, 0:1])
            nc.vector.tensor_copy(scr[:, 1:2], sc_sb[:, 0:1])

            # One persistent PSUM pool; reuse goes through tag rotation so each
            # PE instruction carries at most one sync wait (codegen limit).
            # Banks: plk 2 + plg 2 + pk 2 + pv0/pv1 2 = 8.
            with tc.tile_pool(name="pp", bufs=1, space="PSUM") as pp:
                pv0 = pp.tile([K, C], f32, tag="pv0", bufs=1)
                pv1 = pp.tile([K, 1], f32, tag="pv1", bufs=1)
                # warm-up burst: absorbs the cwt/ones DMA waits (1-wait codegen
                # limit) and keeps the PE HAM busy so phase 1 runs at 2.4 GHz
                dummy = pp.tile([128, K], f32, tag="plg", bufs=2)
                nc.tensor.matmul(dummy[0:64, 0:64], lhsT=cwt_sb[:, 0:64],
                                 rhs=cwt_sb[:, 0:64], start=True, stop=True)
                for _ in range(12):
                    dummy = pp.tile([128, K], f32, tag="plg", bufs=2)
                    nc.tensor.matmul(dummy[0:64, 0:64], lhsT=ones_sb[:, 0:64],
                                     rhs=ones_sb[:, 0:64], start=True, stop=True)
                # phase 1: logits matmuls + exp (scaled by inv_norm).
                # xcl lives in a scoped pool; its space is reused for the
                # xlcn stream afterwards (fresh addresses -> single-wait DMAs)
                with tc.tile_pool(name="xclp", bufs=1) as xclp:
                    xcl_sb = xclp.tile([128, CT * Ls], f32, tag="xcl")
                    xc3 = xcl[:].rearrange("(ct p) l -> p ct l", p=128)
                    xs3 = xcl_sb[:].rearrange("p (ct l) -> p ct l", l=Ls)
                    NB = Ls // 512          # 9.75 -> use 512-blocks + tail
                    DCH = 4                 # dma chunks (early phase-1 start)
                    csz = Ls // DCH         # 1248 columns per chunk, all c-tiles
                    for j in range(DCH):
                        nc.sync.dma_start(
                            xs3[:, :, j * csz:(j + 1) * csz],
                            xc3[:, :, j * csz:(j + 1) * csz],
                        )
                    # logits blocks in [K, L]: stationary conv_wT (64-col
                    # weight loads), x streams as the moving operand
                    nblk = (Ls + 511) // 512
                    touched = set()
                    for b in range(nblk):
                        w = min(512, Ls - b * 512)
                        for j in range((b * 512) // csz,
                                       (b * 512 + w - 1) // csz + 1):
                            if j not in touched:
                                touched.add(j)
                                dj = pp.tile([128, K], f32, tag="plg", bufs=2)
                                nc.tensor.matmul(
                                    dj[0:64, 0:64],
                                    lhsT=xcl_sb[:, j * csz:j * csz + 64],
                                    rhs=xcl_sb[:, j * csz:j * csz + 64],
                                    start=True, stop=True)
                        plk = pp.tile([K, 512], f32, tag="plk", bufs=2)
                        for ct in range(CT):
                            nc.tensor.matmul(
                                plk[:, 0:w],
                                lhsT=cwt_sb[:, ct * K:(ct + 1) * K],
                                rhs=xcl_sb[:, ct * Ls + b * 512:
                                           ct * Ls + b * 512 + w],
                                start=(ct == 0),
                                stop=(ct == CT - 1),
                            )
                        nc.scalar.copy(logkl[:, b * 512:b * 512 + w],
                                       plk[:, 0:w])
                    # transpose to [L-tile, K] and exp with per-location
                    # inv-norm scale; sumexp accumulates for free
                    for t in range(NT):
                        plg = pp.tile([128, K], f32, tag="plg", bufs=2)
                        nc.tensor.transpose(
                            plg[:], logkl[:, t * 128:(t + 1) * 128],
                            idf_sb[0:K, 0:K])
                        nc.scalar.activation(
                            expb[:, t * K:(t + 1) * K], plg[:], AF.Exp,
                            scale=invn_sb[:, t:t + 1],
                            accum_out=sume[:, t:t + 1],
                        )

                # phase 2: batched top-2 keep over the free axis
                e3 = expb[:].rearrange("p (t k) -> p t k", k=K)
                t3 = tmpb[:].rearrange("p (t k) -> p t k", k=K)
                k3 = keeplk[:].rearrange("p (t k) -> p t k", k=K)
                m1bc = m1b[:][:, :, None].broadcast_to([128, NT, K])
                m2bc = m2b[:][:, :, None].broadcast_to([128, NT, K])
                nc.vector.tensor_reduce(m1b[:], e3, axis=AX.X, op=OP.max)
                nc.vector.tensor_tensor(t3, e3, m1bc, op=OP.is_ge)
                nc.vector.scalar_tensor_tensor(
                    t3, t3, -10.0, e3, op0=OP.mult, op1=OP.add)
                nc.vector.tensor_reduce(m2b[:], t3, axis=AX.X, op=OP.max)
                nc.vector.tensor_tensor(k3, e3, m2bc, op=OP.is_ge)
                nc.vector.reciprocal(isum[:], sume[:])
                nc.vector.tensor_mul(scc[:], sc_sb[:], isum[:])

                # phase 3: transpose keep [L,K] -> [K,L] into guarded buffer
                for t in range(NT):
                    pk = pp.tile([K, 128], bf16, tag="pk", bufs=2)
                    nc.tensor.transpose(
                        pk[:], keeplk[:, t * K:(t + 1) * K], id_sb[:])
                    nc.scalar.copy(kb[:, G + t * 128: G + (t + 1) * 128], pk[:])

                # phase 4: separable 3x3 box-sum along flattened L
                # h3s[j] = kb[j] + kb[j+1] + kb[j+2]  (i.e. h[j+1], shifted)
                nc.vector.tensor_add(
                    h3s[:, 0:KBW - 2], kb[:, 0:KBW - 2], kb[:, 2:KBW])
                nc.vector.tensor_add(
                    h3s[:, 0:KBW - 2], h3s[:, 0:KBW - 2], kb[:, 1:KBW - 1])
                # cnt[l] = h[G+l-192] + h[G+l] + h[G+l+192], h[j] = h3s[j-1]
                nc.vector.tensor_add(
                    cntb[:], h3s[:, G - 193:G - 193 + Ls],
                    h3s[:, G + 191:G + 191 + Ls])
                nc.vector.tensor_add(
                    cntb[:], cntb[:], h3s[:, G - 1:G - 1 + Ls])

                # phase 5: transpose cnt back, fuse w2 = (cntT * scc) * exp
                w2 = tmpb
                for t in range(NT):
                    pc = pp.tile([128, K], bf16, tag="plg", bufs=2)
                    nc.tensor.transpose(
                        pc[:], cntb[:, t * 128:(t + 1) * 128], id_sb[:K, :K])
                    nc.vector.scalar_tensor_tensor(
                        w2[:, t * K:(t + 1) * K], pc[:], scc[:, t:t + 1],
                        expb[:, t * K:(t + 1) * K], op0=OP.mult, op1=OP.mult)

                # absorb the w2 DVE wait before the accumulation chain
                dummy2 = pp.tile([128, K], f32, tag="plg", bufs=2)
                nc.tensor.matmul(dummy2[0:64, 0:64], lhsT=w2[:, 0:64],
                                 rhs=w2[:, 0:64], start=True, stop=True)

                # phase 6: VLAD matmul, accumulate [K, C+1] over all l-tiles.
                # Each xlcn wave gets its own buffer (in space freed by xclp)
                # so stream DMAs carry a single sync wait.
                x3 = xlcn[:].rearrange("(a p) c -> p a c", p=128)
                with tc.tile_pool(name="xlc", bufs=1) as xlcp:
                    for w in range((NT + XW - 1) // XW):
                        n = min(XW, NT - w * XW)
                        xt = xlcp.tile([128, XW * (C + 1)], f32, tag=f"xt{w}")
                        nc.sync.dma_start(
                            xt[:, 0:n * (C + 1)].rearrange(
                                "p (a c) -> p a c", c=C + 1),
                            x3[:, w * XW:w * XW + n, :],
                        )
                        for i in range(n):
                            t = w * XW + i
                            lt = w2[:, t * K:(t + 1) * K]
                            nc.tensor.matmul(
                                pv0[:], lhsT=lt,
                                rhs=xt[:, i * (C + 1):i * (C + 1) + C],
                                start=(t == 0), stop=(t == NT - 1))
                            nc.tensor.matmul(
                                pv1[:], lhsT=lt,
                                rhs=xt[:, i * (C + 1) + C:(i + 1) * (C + 1)],
                                start=(t == 0), stop=(t == NT - 1))

                    # phase 7: write this core's [K, C+1] partial sums;
                    # host sums the 8 partials and applies centroid subtraction
                    # and the two L2 normalizations (0.03% of the FLOPs)
                    nc.scalar.copy(vl_sb[:, 0:C], pv0[:])
                    nc.scalar.copy(vl_sb[:, C:C + 1], pv1[:])
                    nc.sync.dma_start(y[:], vl_sb[:])
    n = _prune_waits(nc)
    return nc


def _prune_waits(nc):
    """Drop semaphore waits that are transitively implied by another wait on
    the same instruction.

    The walrus codegen used here allows at most ONE sync wait per
    instruction.  Tile's sem assignment is not transitively minimal: e.g. a
    consumer waits on both a DMA completion and on a PE tick even though the
    DMA itself already waited on that PE tick.  Per-proc completion is
    in-order (engine FIFOs, per-queue DMA), so "sem S reached v" implies all
    waits of every instruction on S's proc with cumulative tick <= v held.
    We compute that closure and greedily delete implied waits.
    """
    insts = [ins for bb in nc.main_func.blocks for ins in bb.instructions]
    # proc name -> ordered [(cumtick, instr)] and instr -> its waits
    proc_events = {}
    waits_of = {}
    for ins in insts:
        si = getattr(ins, "sync_info", None)
        if si is None:
            continue
        ow = list(si.on_wait or [])
        waits_of[id(ins)] = [(w.ant_name, w.wait_value) for w in ow]
        for u in (si.on_update or []):
            if getattr(u, "update_mode", None) not in ("sem-inc", "sem-add-imm"):
                continue
            lst = proc_events.setdefault(u.ant_name, [])
            prev = lst[-1][0] if lst else 0
            lst.append((prev + (u.update_value or 1), ins))

    # holds[(sem, tick_idx)] -> {sem: max_threshold} computed lazily with
    # memoization over prefix positions; iterate to fixpoint.
    import bisect

    def prefix_index(sem, v):
        lst = proc_events.get(sem)
        if not lst:
            return None
        ticks = [t for t, _ in lst]
        i = bisect.bisect_left(ticks, v)
        return i if i < len(lst) else None

    memo = {}

    def holds(sem, v, depth=0):
        """Thresholds guaranteed held once sem >= v."""
        if depth > 6:
            return {}
        i = prefix_index(sem, v)
        if i is None:
            return {}
        key = (sem, i)
        if key in memo:
            return memo[key]
        memo[key] = {}      # cut cycles conservatively
        out = {}
        # Pool (gpsimd) has multiple cores; don't assume in-order there.
        inorder = not sem.startswith("Pool")
        rng = range(i + 1) if inorder else (i,)
        for j in rng:
            _, ins = proc_events[sem][j]
            for (s2, v2) in waits_of.get(id(ins), []):
                if out.get(s2, 0) < v2:
                    out[s2] = v2
                sub = holds(s2, v2, depth + 1)
                for s3, v3 in sub.items():
                    if out.get(s3, 0) < v3:
                        out[s3] = v3
        memo[key] = out
        return out

    # cumulative tick of each instruction on its own update proc
    own_tick = {}
    for sem, lst in proc_events.items():
        for tick, ins in lst:
            own_tick[(id(ins), sem)] = tick

    pruned = 0
    for ins in insts:
        si = getattr(ins, "sync_info", None)
        if si is None or not si.on_wait or len(si.on_wait) < 2:
            continue
        ow = list(si.on_wait)
        kept = list(ow)
        for w in ow:
            if len(kept) == 1:
                break
            # same-queue FIFO: waiting on earlier completions of the very
            # queue this instruction executes on is vacuous (per-queue
            # serial execution); addresses here are disjoint anyway.
            mine = own_tick.get((id(ins), w.ant_name))
            if mine is not None and w.wait_value <= mine - 1:
                kept.remove(w)
                pruned += 1
                continue
            others = [o for o in kept if o is not w]
            for o in others:
                h = holds(o.ant_name, o.wait_value)
                if h.get(w.ant_name, 0) >= w.wait_value:
                    kept.remove(w)
                    pruned += 1
                    break
        si.on_wait = kept
    return pruned


def _host_prep(x, conv_w, centroids):
    from concourse import mybir
    bf16np = mybir.dt.np(mybir.dt.bfloat16)

    x = np.ascontiguousarray(x, dtype=np.float32)
    L = H * W
    norm = np.sqrt((x.astype(np.float64) ** 2).sum(0))
    norm = np.maximum(norm, 1e-12).astype(np.float32)       # [H,W]
    inv_norm = (1.0 / norm).astype(np.float32)
    ii = np.arange(H, dtype=np.float32)
    mi = np.minimum(H - 1 - ii, ii)
    m = np.minimum(mi[:, None], mi[None, :]).astype(np.float32)
    m2 = m * m
    minv = (m2 * m2) * inv_norm                              # [H,W]

    xpad = np.zeros((C, H + 2, W), np.float32)
    xpad[:, 1:H + 1, :] = x
    # transposed layout with norm column, padded rows
    xtn = np.zeros(((H + 2) * W, C + 1), np.float32)
    xtn[W:(H + 1) * W, 0:C] = x.reshape(C, L).T
    xtn[W:(H + 1) * W, C] = norm.reshape(L)
    invn_pad = np.zeros((H + 2) * W, np.float32)
    invn_pad[W:(H + 1) * W] = inv_norm.reshape(L)
    minv_pad = np.zeros((H + 2) * W, np.float32)
    minv_pad[W:(H + 1) * W] = minv.reshape(L)

    cwt = np.ascontiguousarray(conv_w.T, dtype=np.float32)   # [C,K]
    cent = np.ascontiguousarray(centroids, dtype=np.float32)
    identb = np.eye(128, dtype=np.float32).astype(bf16np)
    identf = np.eye(128, dtype=np.float32)
    ones = np.ones((128, 128), np.float32)

    in_maps = []
    for core in range(M):
        r0 = core * RPC
        sl = slice(r0 * W, (r0 + RPC + 2) * W)               # slab in padded coords
        sc0c = minv_pad[sl].copy()
        sc0c[0:W] = 0.0                                      # halo rows contribute 0
        sc0c[(RPC + 1) * W:] = 0.0
        in_maps.append({
            "xcl": np.ascontiguousarray(
                xpad[:, r0:r0 + RPC + 2, :].reshape(C, Ls)),
            "xlcn": np.ascontiguousarray(xtn[sl]),
            "cwt": cwt,
            "cent": cent,
            "sc0": np.ascontiguousarray(sc0c.reshape(NT, 128).T),
            "invn": np.ascontiguousarray(invn_pad[sl].reshape(NT, 128).T.copy()),
            "identb": identb,
            "identf": identf,
            "ones": ones,
        })
    return in_maps


def _ensure_ntff_hook():
    """Install the axon NTFF profile hook if the image's antenv lacks it."""
    import types
    try:
        from antenv.axon_hooks import get_axon_ntff_profile_hook  # noqa: F401
        return
    except ImportError:
        pass
    if "/root/.axon_site" not in sys.path:
        sys.path.insert(0, "/root/.axon_site")
    from trn_agent_boot.trn_boot import _ntff_profile_via_ctypes
    hook = _ntff_profile_via_ctypes("/opt/axon/libaxon_pjrt.so")
    mod = types.ModuleType("antenv.axon_hooks")
    mod.get_axon_ntff_profile_hook = lambda: hook
    mod.set_axon_ntff_profile_hook = lambda h: None
    import antenv
    antenv.axon_hooks = mod
    sys.modules["antenv.axon_hooks"] = mod


def _install_neff_cache():
    """Cache compiled NEFFs across processes, keyed by BIR content hash."""
    import hashlib
    import shutil
    import concourse.bass2jax as b2j

    orig = b2j.compile_bir_kernel
    if getattr(orig, "_neff_cached", False):
        return

    def cached(bir_json, tmpdir, neff_name="file.neff"):
        h = hashlib.sha256(
            bir_json if isinstance(bir_json, bytes) else bir_json.encode()
        ).hexdigest()[:24]
        cdir = "/tmp/neff_cache"
        os.makedirs(cdir, exist_ok=True)
        cpath = os.path.join(cdir, h + ".neff")
        if os.path.exists(cpath):
            dst = os.path.join(tmpdir, neff_name)
            os.makedirs(tmpdir, exist_ok=True)
            shutil.copy(cpath, dst)
            return dst
        out = orig(bir_json, tmpdir, neff_name=neff_name)
        shutil.copy(out, cpath)
        return out

    cached._neff_cached = True
    b2j.compile_bir_kernel = cached


def kernel(x, conv_w, centroids):
    import concourse.bass_utils as bu
    from concourse.bass_utils import run_bass_kernel_spmd
    _install_neff_cache()
    if TRACE:
        _ensure_ntff_hook()
        bu.upload_artifacts = lambda tmpdir: "local://" + tmpdir

    if "nc" not in _CACHE:
        _CACHE["nc"] = _build_nc()
    nc = _CACHE["nc"]
    in_maps = _host_prep(np.asarray(x), np.asarray(conv_w), np.asarray(centroids))
    res = run_bass_kernel_spmd(nc, in_maps, list(range(M)), trace=TRACE)
    _CACHE["last"] = res
    red = np.zeros((K, C + 1), np.float32)
    for r in res.results:
        red += np.asarray(r["y"], dtype=np.float32)
    vlad = red[:, :C] - red[:, C:C + 1] * np.asarray(centroids, np.float32)
    vlad /= np.maximum(np.sqrt((vlad ** 2).sum(1))[:, None], 1e-12)
    v = vlad.reshape(1, K * C)
    v /= np.maximum(np.sqrt((v ** 2).sum()), 1e-12)
    return v.astype(np.float32)



# revision 6
# speedup vs baseline: 2.1964x; 2.1964x over previous
"""NetVLAD (vq_codebook) Trainium2 Bass kernel, 8-way spatially sharded. v2.

Math (same identity as v1, verified vs reference):
  xn = x / ||x||_C per location; logits = conv_w @ xn; soft = softmax_K
  fold(unfold(soft) * top2keep) == soft * cnt, cnt = 3x3 box-sum of the
  per-cluster top-2 indicator (border wrap artifacts killed by the
  (min-dist-to-border)^4 mask). vlad = sa2 @ xn.T - rowsum(sa2) * centroids.

v2 layout strategy (all compute in [L-partition, K-free] layout):
  - x streamed bf16 in BOTH layouts ([C,L] tiled for logits-lhsT, [L,C+1]
    for the VLAD moving operand): halves HBM traffic vs f32.
  - logits computed directly into [l-tile, K] tiles (x tile stationary,
    conv_wT moving) -- no transposes anywhere.
  - 3x3 box-sum of keep done on the PE as 5 banded 128x128 0/1 matmuls
    (cnt[l] = sum_{d} A_d^T @ keep[tile t+d]) -- replaces v1's
    transpose / DVE shift-add / transpose-back pipeline.
  - per-core [K, C+1] partials all-reduced on host (0.03% of FLOPs).

Sharding: H=192 rows split 8 ways (24 rows/core + 1 halo row each side).
"""
import os
import sys

sys.path.insert(0, "/opt/trn_rl_repo")
os.environ.setdefault("MYCRO_LOCAL_CACHE", "1")

import numpy as np

C, H, W, K = 512, 192, 192, 64
M = 8                       # cores
RPC = H // M                # 24 rows per core
Ls = (RPC + 2) * W          # 4992 slab locations (incl. 1 halo row each side)
NT = Ls // 128              # 39 l-tiles
CT = C // 128               # 4 c-tiles
C1 = C + 1                  # x columns + norm column
XTG = [3, 5, 7, 10, 14]     # xtb DMA chunk sizes (tiles), staggered completion
TPC = [13, 13, 13]          # top2 chunk sizes
XVG = [13, 13, 13]          # xvb DMA chunk sizes

TRACE = False               # set by test.py for profiling runs
_CACHE = {}


def _build_nc():
    import concourse.bass as bass
    import concourse.tile as tile
    from concourse import mybir

    f32 = mybir.dt.float32
    bf16 = mybir.dt.bfloat16
    AF = mybir.ActivationFunctionType
    OP = mybir.AluOpType
    AX = mybir.AxisListType

    nc = bass.Bass()
    # host-prepped inputs (see _host_prep for layouts)
    xtb = nc.dram_tensor("xtb", [128, NT * CT * 128], bf16, kind="ExternalInput")
    xvb = nc.dram_tensor("xvb", [128, NT * C1], bf16, kind="ExternalInput")
    cwb = nc.dram_tensor("cwb", [128, CT * K], bf16, kind="ExternalInput")
    shb = nc.dram_tensor("shb", [128, 5 * 128], bf16, kind="ExternalInput")
    cst = nc.dram_tensor("cst", [128, 2 * NT], f32, kind="ExternalInput")
    y = nc.dram_tensor("y", [K, C1], f32, kind="ExternalOutput")

    # chunk boundaries
    xtg = np.cumsum([0] + XTG)          # xtb chunks
    tpg = np.cumsum([0] + TPC)          # top2 chunks
    xvg = np.cumsum([0] + XVG)          # xvb chunks
    # cnt chunks trail top2 chunks by 2 tiles (need keep[t+2])
    cng = [0, int(tpg[1]) - 2, int(tpg[2]) - 2, NT]

    with tile.TileContext(nc) as tc:
        with tc.tile_pool(name="big", bufs=1) as big:
            xtb_sb = big.tile([128, NT * CT * 128], bf16, tag="xtb")
            xvb_sb = big.tile([128, NT * C1], bf16, tag="xvb")
            cwb_sb = big.tile([128, CT * K], bf16, tag="cwb")
            shb_sb = big.tile([128, 5 * 128], bf16, tag="shb")
            cst_sb = big.tile([128, 2 * NT], f32, tag="cst")
            expb = big.tile([128, NT * K], f32, tag="expb")
            tmpb = big.tile([128, NT * K], f32, tag="tmpb")
            keep = big.tile([128, NT * K], bf16, tag="keep")
            cnts = big.tile([128, NT * K], bf16, tag="cnts")
            w2b = big.tile([128, NT * K], bf16, tag="w2b")
            sume = big.tile([128, NT], f32, tag="sume")
            m1b = big.tile([128, NT], f32, tag="m1b")
            m2b = big.tile([128, NT], f32, tag="m2b")
            isum = big.tile([128, NT], f32, tag="isum")
            scc = big.tile([128, NT], f32, tag="scc")
            vl_sb = big.tile([K, C1], f32, tag="vl")
            scr = big.tile([128, 4], f32, tag="scr")

            invn = cst_sb[:, 0:NT]
            sc0 = cst_sb[:, NT:2 * NT]

            # constant DMAs first, then the xtb stream chunks
            nc.sync.dma_start(cwb_sb[:], cwb[:])
            nc.sync.dma_start(shb_sb[:], shb[:])
            nc.sync.dma_start(cst_sb[:], cst[:])
            for g in range(len(XTG)):
                a, b = int(xtg[g]) * CT * 128, int(xtg[g + 1]) * CT * 128
                nc.sync.dma_start(xtb_sb[:, a:b], xtb[:, a:b])

            # single-wait "touch" op absorbing the cst DMA completion so
            # downstream scalar instructions carry at most one sync wait
            nc.scalar.copy(scr[:, 0:1], cst_sb[:, 0:1])

            with tc.tile_pool(name="pp", bufs=1, space="PSUM") as pp:
                pv0 = pp.tile([K, C], f32, tag="pv0", bufs=1)
                pv1 = pp.tile([K, 1], f32, tag="pv1", bufs=1)

                # warm-up burst: absorbs cwb/shb DMA waits on the PE and
                # spins the HAM up before phase 1
                dummy = pp.tile([128, K], f32, tag="dum", bufs=1)
                nc.tensor.matmul(dummy[0:64, 0:64], lhsT=cwb_sb[:, 0:64],
                                 rhs=cwb_sb[:, 0:64], start=True, stop=True)
                dummy = pp.tile([128, K], f32, tag="dum", bufs=1)
                nc.tensor.matmul(dummy[:, 0:64], lhsT=shb_sb[:, 0:128],
                                 rhs=shb_sb[:, 0:64], start=True, stop=True)
                for _ in range(8):
                    dummy = pp.tile([128, K], f32, tag="dum", bufs=1)
                    nc.tensor.matmul(dummy[:, 0:64], lhsT=shb_sb[:, 0:128],
                                     rhs=shb_sb[:, 0:64], start=True, stop=True)

                # ---- phase 1: logits straight into [l-tile, K], then exp.
                # lhsT = x c-chunk (stationary), rhs = conv_wT (moving).
                nvb_emitted = 0
                exp_since = 0

                def emit_exp(t, plg):
                    nc.scalar.activation(
                        expb[:, t * K:(t + 1) * K], plg[:], AF.Exp,
                        scale=invn[:, t:t + 1],
                        accum_out=sume[:, t:t + 1],
                    )

                for g in range(len(XTG)):
                    t0g, t1g = int(xtg[g]), int(xtg[g + 1])
                    # dummy matmul reading this chunk absorbs its DMA wait
                    dummy = pp.tile([128, K], f32, tag="dum", bufs=1)
                    nc.tensor.matmul(
                        dummy[:, 0:64],
                        lhsT=xtb_sb[:, t0g * CT * 128:t0g * CT * 128 + 128],
                        rhs=xtb_sb[:, t0g * CT * 128:t0g * CT * 128 + 64],
                        start=True, stop=True)
                    for t in range(t0g, t1g):
                        plg = pp.tile([128, K], f32, tag="plg", bufs=3)
                        for ct in range(CT):
                            nc.tensor.matmul(
                                plg[:],
                                lhsT=xtb_sb[:, (t * CT + ct) * 128:
                                            (t * CT + ct + 1) * 128],
                                rhs=cwb_sb[:, ct * K:(ct + 1) * K],
                                start=(ct == 0), stop=(ct == CT - 1),
                            )
                        emit_exp(t, plg)
                        exp_since += 1
                        # pace the xvb stream off scalar-engine progress
                        if exp_since in (25, 32, 39) and nvb_emitted < 3:
                            a, b = int(xvg[nvb_emitted]) * C1, \
                                int(xvg[nvb_emitted + 1]) * C1
                            nc.scalar.dma_start(xvb_sb[:, a:b], xvb[:, a:b])
                            nvb_emitted += 1

                # ---- phase 2: batched top-2 keep along K (free axis),
                # one chunk of tiles at a time; scc = sc0 / (norm * sumexp)
                for b in range(len(TPC)):
                    ta, tb = int(tpg[b]), int(tpg[b + 1])
                    n = tb - ta
                    e3 = expb[:, ta * K:tb * K].rearrange(
                        "p (t k) -> p t k", k=K)
                    t3 = tmpb[:, ta * K:tb * K].rearrange(
                        "p (t k) -> p t k", k=K)
                    k3 = keep[:, ta * K:tb * K].rearrange(
                        "p (t k) -> p t k", k=K)
                    m1c = m1b[:, ta:tb][:, :, None].broadcast_to([128, n, K])
                    m2c = m2b[:, ta:tb][:, :, None].broadcast_to([128, n, K])
                    nc.vector.tensor_reduce(
                        m1b[:, ta:tb], e3, axis=AX.X, op=OP.max)
                    nc.vector.tensor_tensor(t3, e3, m1c, op=OP.is_ge)
                    nc.vector.scalar_tensor_tensor(
                        t3, t3, -10.0, e3, op0=OP.mult, op1=OP.add)
                    nc.vector.tensor_reduce(
                        m2b[:, ta:tb], t3, axis=AX.X, op=OP.max)
                    nc.vector.tensor_tensor(k3, e3, m2c, op=OP.is_ge)
                    # scc chunk: reciprocal of sumexp, times mask^4 * invn
                    nc.vector.reciprocal(isum[:, ta:tb], sume[:, ta:tb])
                    nc.vector.tensor_mul(
                        scc[:, ta:tb], sc0[:, ta:tb], isum[:, ta:tb])

                # ---- phase 3: cnt = 3x3 box-sum via 5 banded matmuls,
                # then cnts = cnt * scc on the scalar engine (PSUM drain)
                for b in range(len(TPC)):
                    ca, cb = cng[b], cng[b + 1]
                    # touch matmul absorbs this chunk's keep (DVE) wait
                    dummy = pp.tile([128, K], f32, tag="dum", bufs=1)
                    nc.tensor.matmul(
                        dummy[:, 0:K],
                        lhsT=shb_sb[:, 0:128],
                        rhs=keep[:, ca * K:(ca + 1) * K],
                        start=True, stop=True)
                    # scalar touch absorbs the scc (DVE) wait for this chunk
                    nc.scalar.copy(scr[:, 1:2], scc[:, ca:ca + 1])
                    for t in range(ca, cb):
                        pcn = pp.tile([128, K], f32, tag="pcn", bufs=2)
                        ds = [d for d in range(-2, 3) if 0 <= t + d < NT]
                        for i, d in enumerate(ds):
                            nc.tensor.matmul(
                                pcn[:],
                                lhsT=shb_sb[:, (d + 2) * 128:(d + 3) * 128],
                                rhs=keep[:, (t + d) * K:(t + d + 1) * K],
                                start=(i == 0), stop=(i == len(ds) - 1),
                            )
                        nc.scalar.activation(
                            cnts[:, t * K:(t + 1) * K], pcn[:], AF.Copy,
                            scale=scc[:, t:t + 1])
                    # w2 = cnts * e for this chunk (DVE, writes bf16)
                    nc.vector.tensor_mul(
                        w2b[:, ca * K:cb * K], cnts[:, ca * K:cb * K],
                        expb[:, ca * K:cb * K])

                # ---- phase 4: VLAD accumulation over all l-tiles
                for g in range(len(XVG)):
                    va, vb = int(xvg[g]), int(xvg[g + 1])
                    # dummy absorbs the w2 (DVE) wait so the stream matmuls
                    # carry only the xvb DMA wait
                    dummy = pp.tile([128, K], f32, tag="dum", bufs=1)
                    nc.tensor.matmul(
                        dummy[:, 0:K],
                        lhsT=shb_sb[:, 0:128],
                        rhs=w2b[:, va * K:(va + 1) * K],
                        start=True, stop=True)
                    for t in range(va, vb):
                        lt = w2b[:, t * K:(t + 1) * K]
                        nc.tensor.matmul(
                            pv0[:], lhsT=lt,
                            rhs=xvb_sb[:, t * C1:t * C1 + C],
                            start=(t == 0), stop=(t == NT - 1))
                        nc.tensor.matmul(
                            pv1[:], lhsT=lt,
                            rhs=xvb_sb[:, t * C1 + C:(t + 1) * C1],
                            start=(t == 0), stop=(t == NT - 1))

                # ---- phase 5: drain partials; host sums the 8 cores and
                # applies centroid subtraction + the two L2 normalizations
                nc.scalar.copy(vl_sb[:, 0:C], pv0[:])
                nc.scalar.copy(vl_sb[:, C:C1], pv1[:])
                nc.sync.dma_start(y[:], vl_sb[:])
    _prune_waits(nc)
    return nc


def _prune_waits(nc):
    """Drop semaphore waits transitively implied by another wait on the same
    instruction (the walrus codegen allows one sync wait per instruction).
    Per-proc completion is in-order, so "sem S reached v" implies all waits
    of every instruction on S's proc with cumulative tick <= v held."""
    insts = [ins for bb in nc.main_func.blocks for ins in bb.instructions]
    proc_events = {}
    waits_of = {}
    # per-engine instruction streams (engine queues execute in-order, so a
    # later instruction's completion implies every earlier same-engine
    # instruction's waits held -- including non-sem-updating ones like
    # InstLdweights). GpSimd (Pool) has multiple cores; skip the in-order
    # assumption there.
    stream_of = {}           # id(ins) -> (engine_key, index)
    stream_cum = {}          # engine_key -> list of cumulative wait-unions
    for ins in insts:
        si = getattr(ins, "sync_info", None)
        if si is None:
            continue
        ow = list(si.on_wait or [])
        waits_of[id(ins)] = [(w.ant_name, w.wait_value) for w in ow]
        eng = str(getattr(ins, "engine", None))
        if eng and "Pool" not in eng:
            lst = stream_cum.setdefault(eng, [])
            cur = dict(lst[-1]) if lst else {}
            for (s2, v2) in waits_of[id(ins)]:
                if cur.get(s2, 0) < v2:
                    cur[s2] = v2
            stream_of[id(ins)] = (eng, len(lst))
            lst.append(cur)
        for u in (si.on_update or []):
            if getattr(u, "update_mode", None) not in ("sem-inc", "sem-add-imm"):
                continue
            lst = proc_events.setdefault(u.ant_name, [])
            prev = lst[-1][0] if lst else 0
            lst.append((prev + (u.update_value or 1), ins))

    import bisect

    def holds(sem, v):
        """Exact transitive closure of thresholds implied by sem >= v."""
        out = {}
        work = [(sem, v)]
        while work:
            s, t = work.pop()
            lst = proc_events.get(s)
            if not lst:
                continue
            ticks = [tk for tk, _ in lst]
            i = bisect.bisect_left(ticks, t)
            if i >= len(lst):
                continue
            implied = {}
            for j in range(i + 1):
                _, ins = lst[j]
                st = stream_of.get(id(ins))
                if st is not None:
                    # all same-engine instructions up to ins completed
                    for (s2, v2) in stream_cum[st[0]][st[1]].items():
                        if implied.get(s2, 0) < v2:
                            implied[s2] = v2
                else:
                    for (s2, v2) in waits_of.get(id(ins), []):
                        if implied.get(s2, 0) < v2:
                            implied[s2] = v2
            for s2, v2 in implied.items():
                if out.get(s2, 0) < v2:
                    out[s2] = v2
                    work.append((s2, v2))
        return out

    own_tick = {}
    for sem, lst in proc_events.items():
        for tick, ins in lst:
            own_tick[(id(ins), sem)] = tick

    pruned = 0
    for ins in insts:
        si = getattr(ins, "sync_info", None)
        if si is None or not si.on_wait or len(si.on_wait) < 2:
            continue
        ow = list(si.on_wait)
        kept = list(ow)
        for w in ow:
            if len(kept) == 1:
                break
            mine = own_tick.get((id(ins), w.ant_name))
            if mine is not None and w.wait_value <= mine - 1:
                kept.remove(w)
                pruned += 1
                continue
            others = [o for o in kept if o is not w]
            for o in others:
                h = holds(o.ant_name, o.wait_value)
                if h.get(w.ant_name, 0) >= w.wait_value:
                    kept.remove(w)
                    pruned += 1
                    break
        si.on_wait = kept
    return pruned


def _host_prep(x, conv_w, centroids):
    from concourse import mybir
    bf16np = mybir.dt.np(mybir.dt.bfloat16)

    x = np.ascontiguousarray(x, dtype=np.float32)
    L = H * W
    norm = np.sqrt((x.astype(np.float64) ** 2).sum(0))
    norm = np.maximum(norm, 1e-12).astype(np.float32)        # [H,W]
    inv_norm = (1.0 / norm).astype(np.float32)
    ii = np.arange(H, dtype=np.float32)
    mi = np.minimum(H - 1 - ii, ii)
    m = np.minimum(mi[:, None], mi[None, :]).astype(np.float32)
    m2 = m * m
    minv = (m2 * m2) * inv_norm                              # [H,W]

    xb = x.astype(bf16np)                                    # bf16 once
    xpad = np.zeros((C, H + 2, W), bf16np)
    xpad[:, 1:H + 1, :] = xb
    # transposed layout with norm column, padded rows
    xtn = np.zeros(((H + 2) * W, C1), bf16np)
    xtn[W:(H + 1) * W, 0:C] = xb.reshape(C, L).T
    xtn[W:(H + 1) * W, C] = norm.reshape(L).astype(bf16np)
    invn_pad = np.zeros((H + 2) * W, np.float32)
    invn_pad[W:(H + 1) * W] = inv_norm.reshape(L)
    minv_pad = np.zeros((H + 2) * W, np.float32)
    minv_pad[W:(H + 1) * W] = minv.reshape(L)

    cwb = np.ascontiguousarray(
        conv_w.astype(np.float32).T.reshape(CT, 128, K)
        .transpose(1, 0, 2).reshape(128, CT * K)).astype(bf16np)

    # 5 banded 0/1 shift matrices: A_d[p', p] = 1 iff 128d + p' - p in D
    D = {di * W + dj for di in (-1, 0, 1) for dj in (-1, 0, 1)}
    shb = np.zeros((128, 5 * 128), bf16np)
    pp_, p_ = np.meshgrid(np.arange(128), np.arange(128), indexing="ij")
    for j, d in enumerate(range(-2, 3)):
        band = np.isin(128 * d + pp_ - p_, list(D))
        shb[:, j * 128:(j + 1) * 128] = band.astype(bf16np)

    in_maps = []
    for core in range(M):
        r0 = core * RPC
        sl = slice(r0 * W, (r0 + RPC + 2) * W)               # slab in padded coords
        sc0c = minv_pad[sl].copy()
        sc0c[0:W] = 0.0                                      # halo rows contribute 0
        sc0c[(RPC + 1) * W:] = 0.0
        # xtb: [128, (t, ct, li)] with value x[ct*128+p, t*128+li]
        xs = xpad[:, r0:r0 + RPC + 2, :].reshape(C, Ls)
        xtb = np.ascontiguousarray(
            xs.reshape(CT, 128, NT, 128).transpose(1, 2, 0, 3)
            .reshape(128, NT * CT * 128))
        # xvb: [128, (t, c)] with value xT[t*128+p, c]
        xvb = np.ascontiguousarray(
            xtn[sl].reshape(NT, 128, C1).transpose(1, 0, 2)
            .reshape(128, NT * C1))
        cstc = np.concatenate(
            [invn_pad[sl].reshape(NT, 128).T,
             sc0c.reshape(NT, 128).T], axis=1)
        in_maps.append({
            "xtb": xtb,
            "xvb": xvb,
            "cwb": cwb,
            "shb": shb,
            "cst": np.ascontiguousarray(cstc, dtype=np.float32),
        })
    return in_maps


def _ensure_ntff_hook():
    """Install the axon NTFF profile hook if the image's antenv lacks it."""
    import types
    try:
        from antenv.axon_hooks import get_axon_ntff_profile_hook  # noqa: F401
        return
    except ImportError:
        pass
    if "/root/.axon_site" not in sys.path:
        sys.path.insert(0, "/root/.axon_site")
    from trn_agent_boot.trn_boot import _ntff_profile_via_ctypes
    hook = _ntff_profile_via_ctypes("/opt/axon/libaxon_pjrt.so")
    mod = types.ModuleType("antenv.axon_hooks")
    mod.get_axon_ntff_profile_hook = lambda: hook
    mod.set_axon_ntff_profile_hook = lambda h: None
    import antenv
    antenv.axon_hooks = mod
    sys.modules["antenv.axon_hooks"] = mod


def _install_neff_cache():
    """Cache compiled NEFFs across processes, keyed by BIR content hash."""
    import hashlib
    import shutil
    import concourse.bass2jax as b2j

    orig = b2j.compile_bir_kernel
    if getattr(orig, "_neff_cached", False):
        return

    def cached(bir_json, tmpdir, neff_name="file.neff"):
        h = hashlib.sha256(
            bir_json if isinstance(bir_json, bytes) else bir_json.encode()
        ).hexdigest()[:24]
        cdir = "/tmp/neff_cache"
        os.makedirs(cdir, exist_ok=True)
        cpath = os.path.join(cdir, h + ".neff")
        if os.path.exists(cpath):
            dst = os.path.join(tmpdir, neff_name)
            os.makedirs(tmpdir, exist_ok=True)
            shutil.copy(cpath, dst)
            return dst
        out = orig(bir_json, tmpdir, neff_name=neff_name)
        shutil.copy(out, cpath)
        return out

    cached._neff_cached = True
    b2j.compile_bir_kernel = cached


def kernel(x, conv_w, centroids):
    import concourse.bass_utils as bu
    from concourse.bass_utils import run_bass_kernel_spmd
    _install_neff_cache()
    if TRACE:
        _ensure_ntff_hook()
        bu.upload_artifacts = lambda tmpdir: "local://" + tmpdir

    if "nc" not in _CACHE:
        _CACHE["nc"] = _build_nc()
    nc = _CACHE["nc"]
    in_maps = _host_prep(np.asarray(x), np.asarray(conv_w), np.asarray(centroids))
    res = run_bass_kernel_spmd(nc, in_maps, list(range(M)), trace=TRACE)
    _CACHE["last"] = res
    red = np.zeros((K, C1), np.float32)
    for r in res.results:
        red += np.asarray(r["y"], dtype=np.float32)
    vlad = red[:, :C] - red[:, C:C1] * np.asarray(centroids, np.float32)
    vlad /= np.maximum(np.sqrt((vlad ** 2).sum(1))[:, None], 1e-12)
    v = vlad.reshape(1, K * C)
    v /= np.maximum(np.sqrt((v ** 2).sum()), 1e-12)
    return v.astype(np.float32)


# revision 17
# speedup vs baseline: 2.8663x; 1.3050x over previous
"""NetVLAD (vq_codebook) Trainium2 Bass kernel, 8-way spatially sharded. v3.

Math (same identity as v1/v2):
  xn = x / ||x||_C per location; logits = conv_w @ xn; soft = softmax_K
  fold(unfold(soft) * top2keep) == soft * cnt, cnt = 3x3 box-sum of the
  per-cluster top-2 indicator (border wrap artifacts killed by the
  (min-dist-to-border)^4 mask). vlad = sa2 @ xn.T - rowsum(sa2) * centroids.

v3 strategy (all compute in [L-partition, K-free] layout, no transposes):
  - x is L2-normalized on the host and streamed quantized in BOTH layouts:
    fp8e4m3 [C,L]-tiled for the logits lhsT, bf16 [L,C+1] for the VLAD
    moving operand (+ a ones column that yields rowsum(sa2) for free).
  - logits computed directly into [l-tile, K] PSUM banks, 4 tiles/bank;
    one batched exp per bank (no scale, no accumulator read).
  - softmax sum + top-2 keep on DVE in bf16 chunks; cnt = 3x3 box-sum on
    the PE as 5 banded 128x128 0/1 matmuls per tile, d-outer waves.
  - per-core [K, C+1] partials reduced on host (0.03% of FLOPs).
  - xvb stream chunks are gated behind scalar-engine progress via
    WAR deps (a scalar touch reads the chunk region before the DMA
    writes it) so the Tile scheduler cannot front-run the x load.

Sharding: H=192 rows split 8 ways (24 rows/core + 1 halo row each side).
"""
import os
import sys

sys.path.insert(0, "/opt/trn_rl_repo")
os.environ.setdefault("MYCRO_LOCAL_CACHE", "1")

import numpy as np

C, H, W, K = 512, 192, 192, 64
M = 8                       # cores
RPC = H // M                # 24 rows per core
Ls = (RPC + 2) * W          # 4992 slab locations (incl. 1 halo row each side)
NT = Ls // 128              # 39 l-tiles
CT = C // 128               # 4 c-tiles
C1 = C + 1                  # x columns + ones column
XTG = [3, 5, 7, 10, 14]     # xtb DMA chunk sizes (tiles), staggered completion
TPB = [(0, 17), (17, 34), (34, 39)]        # top2 chunks
CNW = [(1, 14), (14, 27), (27, 38)]        # cnt waves == vlad groups
EB = 4                      # exp batch: tiles per PSUM bank
GATES = {1: 0, 4: 1, 7: 2}  # exp-batch index -> xvb chunk to launch after

TRACE = False               # set by test.py for profiling runs
_CACHE = {}


def _build_nc():
    import concourse.bass as bass
    import concourse.tile as tile
    from concourse import mybir

    f32 = mybir.dt.float32
    bf16 = mybir.dt.bfloat16
    fp8 = mybir.dt.float8e4
    AF = mybir.ActivationFunctionType
    OP = mybir.AluOpType
    AX = mybir.AxisListType

    NV = CNW[-1][1] - CNW[0][0]        # 37 tiles carried in xvb (skip 0, 38)
    V0 = CNW[0][0]

    nc = bass.Bass()
    xtb = nc.dram_tensor("xtb", [128, NT * CT * 128], fp8, kind="ExternalInput")
    xvb = nc.dram_tensor("xvb", [128, NV * C1], bf16, kind="ExternalInput")
    cwb = nc.dram_tensor("cwb", [128, CT * K], fp8, kind="ExternalInput")
    shb = nc.dram_tensor("shb", [128, 5 * 128], bf16, kind="ExternalInput")
    cst = nc.dram_tensor("cst", [128, NT], f32, kind="ExternalInput")
    y = nc.dram_tensor("y", [K, C1], f32, kind="ExternalOutput")

    xtg = np.cumsum([0] + XTG)

    with tile.TileContext(nc) as tc:
        with tc.tile_pool(name="big", bufs=1) as big:
            xtb_sb = big.tile([128, NT * CT * 128], fp8, tag="xtb")
            xvb_sb = big.tile([128, NV * C1], bf16, tag="xvb")
            cwb_sb = big.tile([128, CT * K], fp8, tag="cwb")
            shb_sb = big.tile([128, 5 * 128], bf16, tag="shb")
            sc0 = big.tile([128, NT], f32, tag="sc0")
            expb = big.tile([128, NT * K], bf16, tag="expb")
            tmpb = big.tile([128, NT * K], bf16, tag="tmpb")
            keep = big.tile([128, NT * K], bf16, tag="keep")
            seb = big.tile([128, NT * K], bf16, tag="seb")
            cntb = big.tile([128, NT * K], bf16, tag="cntb")
            w2b = big.tile([128, NT * K], bf16, tag="w2b")
            sume = big.tile([128, NT], f32, tag="sume")
            isum = big.tile([128, NT], f32, tag="isum")
            scc = big.tile([128, NT], f32, tag="scc")
            m1b = big.tile([128, NT], bf16, tag="m1b")
            m2b = big.tile([128, NT], bf16, tag="m2b")
            vl_sb = big.tile([K, C1], f32, tag="vl")
            scr = big.tile([128, 4], f32, tag="scr")

            # constant DMAs first, then the staggered xtb stream
            nc.sync.dma_start(cwb_sb[:], cwb[:])
            nc.sync.dma_start(shb_sb[:], shb[:])
            nc.sync.dma_start(sc0[:], cst[:])
            for g in range(len(XTG)):
                a, b = int(xtg[g]) * CT * 128, int(xtg[g + 1]) * CT * 128
                nc.sync.dma_start(xtb_sb[:, a:b], xtb[:, a:b])

            # single-wait touch absorbing the cst DMA for later DVE/scalar use
            nc.scalar.copy(scr[:, 0:1], sc0[:, 0:1])
            # init the xvb gate columns so the gate touches read defined data
            for (va, _vb) in CNW:
                a = (va - V0) * C1
                nc.vector.memset(xvb_sb[:, a:a + 1], 0.0)

            with tc.tile_pool(name="pp", bufs=1, space="PSUM") as pp:
                # all PSUM tiles are whole-bank (2048B) multiples so every
                # allocation is bank-aligned and matmuls never cross banks:
                # pcn 2 + pv0 1 + pv1 1 + plg 3 + dum 1 = 8 banks
                pcn = pp.tile([128, 2 * C], f32, tag="pcn", bufs=1)
                pv0 = pp.tile([K, C], f32, tag="pv0", bufs=1)
                pv1 = pp.tile([K, C], f32, tag="pv1", bufs=1)
                plg0 = pp.tile([128, C], f32, tag="plg", bufs=3,
                               name="plg0")  # reserve bank-aligned slots

                # warm-up burst: absorbs cwb/shb DMA waits, spins up the HAM
                dummy = pp.tile([128, K], f32, tag="dum", bufs=1)
                nc.tensor.matmul(dummy[0:64, 0:64], lhsT=cwb_sb[:, 0:64],
                                 rhs=cwb_sb[:, 0:64], start=True, stop=True)
                for _ in range(6):
                    dummy = pp.tile([128, K], f32, tag="dum", bufs=1)
                    nc.tensor.matmul(dummy[:, 0:64], lhsT=shb_sb[:, 0:128],
                                     rhs=shb_sb[:, 0:64], start=True, stop=True)

                # ---- phase 1: logits into [l-tile, K] PSUM, 4 tiles/bank;
                # batched exp per bank (x pre-normalized, so no scale)
                nb = (NT + EB - 1) // EB
                gate_cols = {}
                plogs = {}
                cg = 0                      # xtb chunk cursor

                for bi in range(nb):
                    t0, t1 = bi * EB, min(NT, (bi + 1) * EB)
                    plg = pp.tile([128, C], f32, tag="plg", bufs=3)
                    plogs[bi] = plg
                    for t in range(t0, t1):
                        if cg < len(XTG) and t == int(xtg[cg]):
                            # dummy matmul absorbs this xtb chunk's DMA wait
                            dummy = pp.tile([128, K], f32, tag="dum", bufs=1)
                            base = t * CT * 128
                            nc.tensor.matmul(
                                dummy[:, 0:64],
                                lhsT=xtb_sb[:, base:base + 128],
                                rhs=xtb_sb[:, base:base + 64],
                                start=True, stop=True)
                            cg += 1
                        for ct in range(CT):
                            nc.tensor.matmul(
                                plg[:, (t - t0) * K:(t - t0 + 1) * K],
                                lhsT=xtb_sb[:, (t * CT + ct) * 128:
                                            (t * CT + ct + 1) * 128],
                                rhs=cwb_sb[:, ct * K:(ct + 1) * K],
                                start=(ct == 0), stop=(ct == CT - 1),
                            )
                    nc.scalar.activation(
                        expb[:, t0 * K:t1 * K], plg[:, 0:(t1 - t0) * K],
                        AF.Exp)
                    if bi in GATES:
                        # gate: touch-read the xvb chunk region, then issue
                        # its DMA; the WAR dep paces the stream off scalar
                        # progress (the Tile scheduler keeps DMA after touch)
                        gi = GATES[bi]
                        va, vb = CNW[gi]
                        a = (va - V0) * C1
                        b = (vb - V0) * C1
                        nc.scalar.copy(scr[:, 2:3], xvb_sb[:, a:a + 1])
                        nc.sync.dma_start(xvb_sb[:, a:b], xvb[:, a:b])

                # ---- phase 2: per chunk: softmax sum, top-2 keep, and
                # se = e * mask^4 / sumexp (all DVE; e is bf16)
                for (ta, tb) in TPB:
                    n = tb - ta
                    e3 = expb[:, ta * K:tb * K].rearrange(
                        "p (t k) -> p t k", k=K)
                    t3 = tmpb[:, ta * K:tb * K].rearrange(
                        "p (t k) -> p t k", k=K)
                    k3 = keep[:, ta * K:tb * K].rearrange(
                        "p (t k) -> p t k", k=K)
                    s3 = seb[:, ta * K:tb * K].rearrange(
                        "p (t k) -> p t k", k=K)
                    m1c = m1b[:, ta:tb][:, :, None].broadcast_to([128, n, K])
                    m2c = m2b[:, ta:tb][:, :, None].broadcast_to([128, n, K])
                    scc_c = scc[:, ta:tb][:, :, None].broadcast_to([128, n, K])
                    nc.vector.tensor_reduce(
                        m1b[:, ta:tb], e3, axis=AX.X, op=OP.max)
                    nc.vector.tensor_tensor(t3, e3, m1c, op=OP.is_ge)
                    nc.vector.scalar_tensor_tensor(
                        t3, t3, -10.0, e3, op0=OP.mult, op1=OP.add)
                    nc.vector.tensor_reduce(
                        m2b[:, ta:tb], t3, axis=AX.X, op=OP.max)
                    nc.vector.tensor_tensor(k3, e3, m2c, op=OP.is_ge)
                    nc.vector.tensor_reduce(
                        sume[:, ta:tb], e3, axis=AX.X, op=OP.add)
                    nc.vector.reciprocal(isum[:, ta:tb], sume[:, ta:tb])
                    nc.vector.tensor_mul(
                        scc[:, ta:tb], sc0[:, ta:tb], isum[:, ta:tb])
                    nc.vector.tensor_tensor(s3, e3, scc_c, op=OP.mult)

                # ---- phases 3+4 interleaved per wave: cnt via banded
                # matmuls (d-outer), scalar PSUM drain, w2 = cnt * se (DVE),
                # then this wave's VLAD matmuls (gated by its xvb chunk)
                for wi, (wa, wb) in enumerate(CNW):
                    # dummy absorbs this wave's keep (DVE) wait
                    dummy = pp.tile([128, K], f32, tag="dum", bufs=1)
                    nc.tensor.matmul(
                        dummy[:, 0:K],
                        lhsT=shb_sb[:, 0:128],
                        rhs=keep[:, wa * K:(wa + 1) * K],
                        start=True, stop=True)
                    # d-outer banded matmuls: one LDWEIGHTS per shift matrix.
                    # start/stop once per 2KB PSUM zero region (8 tiles/bank):
                    # start=True lazily zeroes the bank; first-touch writes of
                    # other tiles store, later touches accumulate.
                    pairs = [(d, t) for d in range(-2, 3)
                             for t in range(wa, wb) if 0 <= t + d < NT]
                    bank = lambda t: (t - wa) // 8
                    fidx = {}
                    lidx = {}
                    for i, (d, t) in enumerate(pairs):
                        fidx.setdefault(bank(t), i)
                        lidx[bank(t)] = i
                    for i, (d, t) in enumerate(pairs):
                        nc.tensor.matmul(
                            pcn[:, (t - wa) * K:(t - wa + 1) * K],
                            lhsT=shb_sb[:, (d + 2) * 128:(d + 3) * 128],
                            rhs=keep[:, (t + d) * K:(t + d + 1) * K],
                            start=(i == fidx[bank(t)]),
                            stop=(i == lidx[bank(t)]),
                        )
                    # drain wave PSUM -> bf16 cnt, 4-ish tiles per copy
                    for t in range(wa, wb, EB):
                        te = min(wb, t + EB)
                        nc.scalar.copy(
                            cntb[:, t * K:te * K],
                            pcn[:, (t - wa) * K:(te - wa) * K])
                    # w2 = cnt * se for the wave (DVE, bf16 2x)
                    nc.vector.tensor_mul(
                        w2b[:, wa * K:wb * K], cntb[:, wa * K:wb * K],
                        seb[:, wa * K:wb * K])
                    # VLAD for the wave; dummy absorbs the w2 DVE wait so
                    # the stream matmuls carry only the xvb DMA wait
                    dummy = pp.tile([128, K], f32, tag="dum", bufs=1)
                    nc.tensor.matmul(
                        dummy[:, 0:K],
                        lhsT=shb_sb[:, 0:128],
                        rhs=w2b[:, wa * K:(wa + 1) * K],
                        start=True, stop=True)
                    for t in range(wa, wb):
                        lt = w2b[:, t * K:(t + 1) * K]
                        base = (t - V0) * C1
                        nc.tensor.matmul(
                            pv0[:], lhsT=lt,
                            rhs=xvb_sb[:, base:base + C],
                            start=(t == CNW[0][0]), stop=(t == CNW[-1][1] - 1))
                        nc.tensor.matmul(
                            pv1[:, 0:1], lhsT=lt,
                            rhs=xvb_sb[:, base + C:base + C1],
                            start=(t == CNW[0][0]), stop=(t == CNW[-1][1] - 1))

                # ---- drain partials; host sums cores + normalizes
                nc.scalar.copy(vl_sb[:, 0:C], pv0[:])
                nc.scalar.copy(vl_sb[:, C:C1], pv1[:, 0:1])
                nc.sync.dma_start(y[:], vl_sb[:])
    _prune_waits(nc)
    return nc


def _prune_waits(nc):
    """Drop semaphore waits transitively implied by another wait on the same
    instruction (the walrus codegen allows one sync wait per instruction).
    Per-engine queues execute in-order, so a later instruction's completion
    implies every earlier same-engine instruction's waits held (including
    non-sem-updating ones like InstLdweights)."""
    insts = [ins for bb in nc.main_func.blocks for ins in bb.instructions]
    proc_events = {}
    waits_of = {}
    stream_of = {}           # id(ins) -> (engine_key, index)
    stream_cum = {}          # engine_key -> list of cumulative wait-unions
    for ins in insts:
        si = getattr(ins, "sync_info", None)
        if si is None:
            continue
        ow = list(si.on_wait or [])
        waits_of[id(ins)] = [(w.ant_name, w.wait_value) for w in ow]
        eng = str(getattr(ins, "engine", None))
        if eng and "Pool" not in eng:
            lst = stream_cum.setdefault(eng, [])
            cur = dict(lst[-1]) if lst else {}
            for (s2, v2) in waits_of[id(ins)]:
                if cur.get(s2, 0) < v2:
                    cur[s2] = v2
            stream_of[id(ins)] = (eng, len(lst))
            lst.append(cur)
        for u in (si.on_update or []):
            if getattr(u, "update_mode", None) not in ("sem-inc", "sem-add-imm"):
                continue
            lst = proc_events.setdefault(u.ant_name, [])
            prev = lst[-1][0] if lst else 0
            lst.append((prev + (u.update_value or 1), ins))

    import bisect

    def holds(sem, v):
        """Exact transitive closure of thresholds implied by sem >= v."""
        out = {}
        work = [(sem, v)]
        while work:
            s, t = work.pop()
            lst = proc_events.get(s)
            if not lst:
                continue
            ticks = [tk for tk, _ in lst]
            i = bisect.bisect_left(ticks, t)
            if i >= len(lst):
                continue
            implied = {}
            for j in range(i + 1):
                _, ins = lst[j]
                st = stream_of.get(id(ins))
                if st is not None:
                    for (s2, v2) in stream_cum[st[0]][st[1]].items():
                        if implied.get(s2, 0) < v2:
                            implied[s2] = v2
                else:
                    for (s2, v2) in waits_of.get(id(ins), []):
                        if implied.get(s2, 0) < v2:
                            implied[s2] = v2
            for s2, v2 in implied.items():
                if out.get(s2, 0) < v2:
                    out[s2] = v2
                    work.append((s2, v2))
        return out

    own_tick = {}
    for sem, lst in proc_events.items():
        for tick, ins in lst:
            own_tick[(id(ins), sem)] = tick

    pruned = 0
    for ins in insts:
        si = getattr(ins, "sync_info", None)
        if si is None or not si.on_wait or len(si.on_wait) < 2:
            continue
        ow = list(si.on_wait)
        kept = list(ow)
        for w in ow:
            if len(kept) == 1:
                break
            mine = own_tick.get((id(ins), w.ant_name))
            if mine is not None and w.wait_value <= mine - 1:
                kept.remove(w)
                pruned += 1
                continue
            others = [o for o in kept if o is not w]
            for o in others:
                h = holds(o.ant_name, o.wait_value)
                if h.get(w.ant_name, 0) >= w.wait_value:
                    kept.remove(w)
                    pruned += 1
                    break
        si.on_wait = kept
    return pruned


def _host_prep(x, conv_w, centroids):
    from concourse import mybir
    bf16np = mybir.dt.np(mybir.dt.bfloat16)
    fp8np = mybir.dt.np(mybir.dt.float8e4)

    x = np.ascontiguousarray(x, dtype=np.float32)
    L = H * W
    norm = np.sqrt((x.astype(np.float64) ** 2).sum(0))
    inv_norm = (1.0 / np.maximum(norm, 1e-12)).astype(np.float32)  # [H,W]
    xn = x * inv_norm[None]                                  # normalized x
    ii = np.arange(H, dtype=np.float32)
    mi = np.minimum(H - 1 - ii, ii)
    m = np.minimum(mi[:, None], mi[None, :]).astype(np.float32)
    m2 = m * m
    mask4 = m2 * m2                                          # [H,W]

    xpad8 = np.zeros((C, H + 2, W), fp8np)
    xpad8[:, 1:H + 1, :] = xn.astype(fp8np)
    xnb_pad = np.zeros(((H + 2) * W, C), bf16np)             # [Lpad, C] bf16
    xnb_pad[W:(H + 1) * W] = xn.reshape(C, L).T.astype(bf16np)
    sc_pad = np.zeros((H + 2) * W, np.float32)
    sc_pad[W:(H + 1) * W] = mask4.reshape(L)

    cwb = np.ascontiguousarray(
        conv_w.astype(np.float32).T.reshape(CT, 128, K)
        .transpose(1, 0, 2).reshape(128, CT * K)).astype(fp8np)

    # 5 banded 0/1 shift matrices: A_d[p', p] = 1 iff 128d + p' - p in D
    D = {di * W + dj for di in (-1, 0, 1) for dj in (-1, 0, 1)}
    shb = np.zeros((128, 5 * 128), bf16np)
    pp_, p_ = np.meshgrid(np.arange(128), np.arange(128), indexing="ij")
    for j, d in enumerate(range(-2, 3)):
        band = np.isin(128 * d + pp_ - p_, list(D))
        shb[:, j * 128:(j + 1) * 128] = band.astype(bf16np)

    V0, V1 = CNW[0][0], CNW[-1][1]
    NV = V1 - V0

    in_maps = []
    for core in range(M):
        r0 = core * RPC
        sl = slice(r0 * W, (r0 + RPC + 2) * W)               # slab, padded coords
        sc0c = sc_pad[sl].copy()
        sc0c[0:W] = 0.0                                      # halo rows give 0
        sc0c[(RPC + 1) * W:] = 0.0
        # xtb: [128, (t, ct, li)] = xn[ct*128+p, t*128+li] (fp8)
        xs = xpad8[:, r0:r0 + RPC + 2, :].reshape(C, Ls)
        xtb = np.ascontiguousarray(
            xs.reshape(CT, 128, NT, 128).transpose(1, 2, 0, 3)
            .reshape(128, NT * CT * 128))
        # xvb: [128, (t-V0, c)] = xnT[t*128+p, c], ones column; tiles V0..V1
        lo = r0 * W + V0 * 128                               # padded coords
        xv = np.ones((NV * 128, C1), bf16np)
        xv[:, 0:C] = xnb_pad[lo:lo + NV * 128]
        xvb = np.ascontiguousarray(
            xv.reshape(NV, 128, C1).transpose(1, 0, 2).reshape(128, NV * C1))
        in_maps.append({
            "xtb": xtb,
            "xvb": xvb,
            "cwb": cwb,
            "shb": shb,
            "cst": np.ascontiguousarray(sc0c.reshape(NT, 128).T,
                                        dtype=np.float32),
        })
    return in_maps


def _ensure_ntff_hook():
    """Install the axon NTFF profile hook if the image's antenv lacks it."""
    import types
    try:
        from antenv.axon_hooks import get_axon_ntff_profile_hook  # noqa: F401
        return
    except ImportError:
        pass
    if "/root/.axon_site" not in sys.path:
        sys.path.insert(0, "/root/.axon_site")
    from trn_agent_boot.trn_boot import _ntff_profile_via_ctypes
    hook = _ntff_profile_via_ctypes("/opt/axon/libaxon_pjrt.so")
    mod = types.ModuleType("antenv.axon_hooks")
    mod.get_axon_ntff_profile_hook = lambda: hook
    mod.set_axon_ntff_profile_hook = lambda h: None
    import antenv
    antenv.axon_hooks = mod
    sys.modules["antenv.axon_hooks"] = mod


def _install_neff_cache():
    """Cache compiled NEFFs across processes, keyed by BIR content hash."""
    import hashlib
    import shutil
    import concourse.bass2jax as b2j

    orig = b2j.compile_bir_kernel
    if getattr(orig, "_neff_cached", False):
        return

    def cached(bir_json, tmpdir, neff_name="file.neff"):
        h = hashlib.sha256(
            bir_json if isinstance(bir_json, bytes) else bir_json.encode()
        ).hexdigest()[:24]
        cdir = "/tmp/neff_cache"
        os.makedirs(cdir, exist_ok=True)
        cpath = os.path.join(cdir, h + ".neff")
        if os.path.exists(cpath):
            dst = os.path.join(tmpdir, neff_name)
            os.makedirs(tmpdir, exist_ok=True)
            shutil.copy(cpath, dst)
            return dst
        out = orig(bir_json, tmpdir, neff_name=neff_name)
        shutil.copy(out, cpath)
        return out

    cached._neff_cached = True
    b2j.compile_bir_kernel = cached


def kernel(x, conv_w, centroids):
    import concourse.bass_utils as bu
    from concourse.bass_utils import run_bass_kernel_spmd
    _install_neff_cache()
    if TRACE:
        _ensure_ntff_hook()
        bu.upload_artifacts = lambda tmpdir: "local://" + tmpdir

    if "nc" not in _CACHE:
        _CACHE["nc"] = _build_nc()
    nc = _CACHE["nc"]
    in_maps = _host_prep(np.asarray(x), np.asarray(conv_w), np.asarray(centroids))
    res = run_bass_kernel_spmd(nc, in_maps, list(range(M)), trace=TRACE)
    _CACHE["last"] = res
    red = np.zeros((K, C1), np.float32)
    for r in res.results:
        red += np.asarray(r["y"], dtype=np.float32)
    vlad = red[:, :C] - red[:, C:C1] * np.asarray(centroids, np.float32)
    vlad /= np.maximum(np.sqrt((vlad ** 2).sum(1))[:, None], 1e-12)
    v = vlad.reshape(1, K * C)
    v /= np.maximum(np.sqrt((v ** 2).sum()), 1e-12)
    return v.astype(np.float32)


# revision 21
# speedup vs baseline: 3.1045x; 1.0831x over previous
"""NetVLAD (vq_codebook) Trainium2 Bass kernel, 8-way spatially sharded. v3.

Math (same identity as v1/v2):
  xn = x / ||x||_C per location; logits = conv_w @ xn; soft = softmax_K
  fold(unfold(soft) * top2keep) == soft * cnt, cnt = 3x3 box-sum of the
  per-cluster top-2 indicator (border wrap artifacts killed by the
  (min-dist-to-border)^4 mask). vlad = sa2 @ xn.T - rowsum(sa2) * centroids.

v3 strategy (all compute in [L-partition, K-free] layout, no transposes):
  - x is L2-normalized on the host and streamed quantized in BOTH layouts:
    fp8e4m3 [C,L]-tiled for the logits lhsT, bf16 [L,C+1] for the VLAD
    moving operand (+ a ones column that yields rowsum(sa2) for free).
  - logits computed directly into [l-tile, K] PSUM banks, 4 tiles/bank;
    one batched exp per bank (no scale, no accumulator read).
  - softmax sum + top-2 keep on DVE in bf16 chunks; cnt = 3x3 box-sum on
    the PE as 5 banded 128x128 0/1 matmuls per tile, d-outer waves.
  - per-core [K, C+1] partials reduced on host (0.03% of FLOPs).
  - xvb stream chunks are gated behind scalar-engine progress via
    WAR deps (a scalar touch reads the chunk region before the DMA
    writes it) so the Tile scheduler cannot front-run the x load.

Sharding: H=192 rows split 8 ways (24 rows/core + 1 halo row each side).
"""
import os
import sys

sys.path.insert(0, "/opt/trn_rl_repo")
os.environ.setdefault("MYCRO_LOCAL_CACHE", "1")

import numpy as np

C, H, W, K = 512, 192, 192, 64
M = 8                       # cores
RPC = H // M                # 24 rows per core
Ls = (RPC + 2) * W          # 4992 slab locations (incl. 1 halo row each side)
NT = Ls // 128              # 39 l-tiles
CT = C // 128               # 4 c-tiles
C1 = C + 1                  # x columns + ones column
XTG = [6, 9, 11, 13]        # xtb DMA chunk sizes (tiles), staggered completion
TPB = [(0, 10), (10, 20), (20, 30), (30, 39)]   # top2 chunks
CNW = [(1, 8), (8, 18), (18, 28), (28, 38)]     # cnt waves == vlad groups
EB = 4                      # exp batch: tiles per PSUM bank
GATES = {1: 0, 3: 1, 5: 2, 7: 3}  # exp-batch index -> xvb chunk after it

TRACE = False               # set by test.py for profiling runs
_CACHE = {}


def _build_nc():
    import concourse.bass as bass
    import concourse.tile as tile
    from concourse import mybir

    f32 = mybir.dt.float32
    bf16 = mybir.dt.bfloat16
    fp8 = mybir.dt.float8e4
    AF = mybir.ActivationFunctionType
    OP = mybir.AluOpType
    AX = mybir.AxisListType

    NV = CNW[-1][1] - CNW[0][0]        # 37 tiles carried in xvb (skip 0, 38)
    V0 = CNW[0][0]

    nc = bass.Bass()
    xtb = nc.dram_tensor("xtb", [128, NT * CT * 128], fp8, kind="ExternalInput")
    xvb = nc.dram_tensor("xvb", [128, NV * C1], bf16, kind="ExternalInput")
    cwb = nc.dram_tensor("cwb", [128, CT * K], fp8, kind="ExternalInput")
    shb = nc.dram_tensor("shb", [128, 5 * 128], bf16, kind="ExternalInput")
    cst = nc.dram_tensor("cst", [128, NT], f32, kind="ExternalInput")
    y = nc.dram_tensor("y", [K, C1], f32, kind="ExternalOutput")

    xtg = np.cumsum([0] + XTG)

    with tile.TileContext(nc) as tc:
        with tc.tile_pool(name="big", bufs=1) as big:
            xtb_sb = big.tile([128, NT * CT * 128], fp8, tag="xtb")
            xvb_sb = big.tile([128, NV * C1], bf16, tag="xvb")
            cwb_sb = big.tile([128, CT * K], fp8, tag="cwb")
            shb_sb = big.tile([128, 5 * 128], bf16, tag="shb")
            sc0 = big.tile([128, NT], f32, tag="sc0")
            expb = big.tile([128, NT * K], bf16, tag="expb")
            keep = big.tile([128, NT * K], bf16, tag="keep")
            seb = big.tile([128, NT * K], bf16, tag="seb")
            cntb = big.tile([128, NT * K], bf16, tag="cntb")
            w2b = big.tile([128, NT * K], bf16, tag="w2b")
            sume = big.tile([128, NT], f32, tag="sume")
            isum = big.tile([128, NT], f32, tag="isum")
            scc = big.tile([128, NT], f32, tag="scc")
            m8 = big.tile([128, NT * 8], bf16, tag="m8")
            vl_sb = big.tile([K, C1], f32, tag="vl")
            scr = big.tile([128, 4], f32, tag="scr")

            # constants go through the scalar HWDGE so the sync DGE can start
            # generating the xtb stream descriptors immediately
            nc.scalar.dma_start(cwb_sb[:], cwb[:])
            nc.scalar.dma_start(shb_sb[:], shb[:])
            nc.scalar.dma_start(sc0[:], cst[:])
            for g in range(len(XTG)):
                a, b = int(xtg[g]) * CT * 128, int(xtg[g + 1]) * CT * 128
                nc.sync.dma_start(xtb_sb[:, a:b], xtb[:, a:b])

            # single-wait touch absorbing the cst DMA for later DVE/scalar use
            nc.scalar.copy(scr[:, 0:1], sc0[:, 0:1])
            # init the xvb gate columns so the gate touches read defined data
            for (va, _vb) in CNW:
                a = (va - V0) * C1
                nc.vector.memset(xvb_sb[:, a:a + 1], 0.0)

            with tc.tile_pool(name="pp", bufs=1, space="PSUM") as pp:
                # all PSUM tiles are whole-bank (2048B) multiples so every
                # allocation is bank-aligned and matmuls never cross banks:
                # pcn 2 + pv0 1 + pv1 1 + plg 3 + dum 1 = 8 banks
                pcn = pp.tile([128, 2 * C], f32, tag="pcn", bufs=1)
                pv0 = pp.tile([K, C], f32, tag="pv0", bufs=1)
                pv1 = pp.tile([K, C], f32, tag="pv1", bufs=1)
                plg0 = pp.tile([128, C], f32, tag="plg", bufs=3,
                               name="plg0")  # reserve bank-aligned slots

                # warm-up burst: absorbs cwb/shb DMA waits, spins up the HAM
                dummy = pp.tile([128, K], f32, tag="dum", bufs=1)
                nc.tensor.matmul(dummy[0:64, 0:64], lhsT=cwb_sb[:, 0:64],
                                 rhs=cwb_sb[:, 0:64], start=True, stop=True)
                for _ in range(6):
                    dummy = pp.tile([128, K], f32, tag="dum", bufs=1)
                    nc.tensor.matmul(dummy[:, 0:64], lhsT=shb_sb[:, 0:128],
                                     rhs=shb_sb[:, 0:64], start=True, stop=True)

                # ---- phase 1: logits into [l-tile, K] PSUM, 4 tiles/bank;
                # batched exp per bank (x pre-normalized, so no scale)
                nb = (NT + EB - 1) // EB
                gate_cols = {}
                plogs = {}
                cg = 0                      # xtb chunk cursor

                DR = mybir.MatmulPerfMode.DoubleRow
                for bi in range(nb):
                    t0, t1 = bi * EB, min(NT, (bi + 1) * EB)
                    plg = pp.tile([128, C], f32, tag="plg", bufs=3)
                    plogs[bi] = plg
                    for t in range(t0, t1):
                        if cg < len(XTG) and t == int(xtg[cg]):
                            # dummy matmul absorbs this xtb chunk's DMA wait
                            dummy = pp.tile([128, K], f32, tag="dum", bufs=1)
                            base = t * CT * 128
                            nc.tensor.matmul(
                                dummy[:, 0:64],
                                lhsT=xtb_sb[:, base:base + 128],
                                rhs=xtb_sb[:, base:base + 64],
                                start=True, stop=True)
                            cg += 1
                        # fp8 DoubleRow: 256-deep contraction per matmul,
                        # c-halves stacked in the AP middle dim
                        for h in range(2):
                            lv = xtb_sb[:, (t * CT + 2 * h) * 128:
                                        (t * CT + 2 * h + 2) * 128]
                            rv = cwb_sb[:, 2 * h * K:(2 * h + 2) * K]
                            nc.tensor.matmul(
                                plg[:, (t - t0) * K:(t - t0 + 1) * K],
                                lhsT=lv.rearrange("p (two f) -> p two f",
                                                  two=2),
                                rhs=rv.rearrange("p (two f) -> p two f",
                                                 two=2),
                                start=(h == 0), stop=(h == 1),
                                perf_mode=DR,
                            )
                    nc.scalar.activation(
                        expb[:, t0 * K:t1 * K], plg[:, 0:(t1 - t0) * K],
                        AF.Exp)
                    if bi in GATES:
                        # gate: touch-read the xvb chunk region, then issue
                        # its DMA; the WAR dep paces the stream off scalar
                        # progress (the Tile scheduler keeps DMA after touch)
                        gi = GATES[bi]
                        va, vb = CNW[gi]
                        a = (va - V0) * C1
                        b = (vb - V0) * C1
                        nc.scalar.copy(scr[:, 2:3], xvb_sb[:, a:a + 1])
                        nc.sync.dma_start(xvb_sb[:, a:b], xvb[:, a:b])

                # ---- phases 2-4, software-pipelined: top2 chunk ci, then
                # wave ci-1's [cnt matmuls -> scalar drain -> w2 -> VLAD]
                # (wave i only needs keep from chunk i, so trailing by one
                # chunk keeps every engine busy without stalls)

                def top2_chunk(ci):
                    ta, tb = TPB[ci]
                    n = tb - ta
                    e3 = expb[:, ta * K:tb * K].rearrange(
                        "p (t k) -> p t k", k=K)
                    k3 = keep[:, ta * K:tb * K].rearrange(
                        "p (t k) -> p t k", k=K)
                    s3 = seb[:, ta * K:tb * K].rearrange(
                        "p (t k) -> p t k", k=K)
                    # per-tile top-8 (Max8); m2 = column 1
                    for t in range(ta, tb):
                        nc.vector.max(m8[:, t * 8:(t + 1) * 8],
                                      expb[:, t * K:(t + 1) * K])
                    m83 = m8[:, ta * 8:tb * 8].rearrange(
                        "p (t e) -> p t e", e=8)
                    m2c = m83[:, :, 1:2].broadcast_to([128, n, K])
                    scc_c = scc[:, ta:tb][:, :, None].broadcast_to([128, n, K])
                    nc.vector.tensor_tensor(k3, e3, m2c, op=OP.is_ge)
                    nc.vector.tensor_reduce(
                        sume[:, ta:tb], e3, axis=AX.X, op=OP.add)
                    nc.vector.reciprocal(isum[:, ta:tb], sume[:, ta:tb])
                    nc.vector.tensor_mul(
                        scc[:, ta:tb], sc0[:, ta:tb], isum[:, ta:tb])
                    nc.vector.tensor_tensor(s3, e3, scc_c, op=OP.mult)

                def wave(wi):
                    wa, wb = CNW[wi]
                    # dummy absorbs this wave's keep (DVE) wait
                    dummy = pp.tile([128, K], f32, tag="dum", bufs=1)
                    nc.tensor.matmul(
                        dummy[:, 0:K],
                        lhsT=shb_sb[:, 0:128],
                        rhs=keep[:, wa * K:(wa + 1) * K],
                        start=True, stop=True)
                    # d-outer banded matmuls: one LDWEIGHTS per shift matrix.
                    # start/stop once per 2KB PSUM zero region (8 tiles/bank):
                    # start=True lazily zeroes the bank; first-touch writes of
                    # other tiles store, later touches accumulate.
                    pairs = [(d, t) for d in range(-2, 3)
                             for t in range(wa, wb) if 0 <= t + d < NT]
                    bank = lambda t: (t - wa) // 8
                    fidx = {}
                    lidx = {}
                    for i, (d, t) in enumerate(pairs):
                        fidx.setdefault(bank(t), i)
                        lidx[bank(t)] = i
                    for i, (d, t) in enumerate(pairs):
                        nc.tensor.matmul(
                            pcn[:, (t - wa) * K:(t - wa + 1) * K],
                            lhsT=shb_sb[:, (d + 2) * 128:(d + 3) * 128],
                            rhs=keep[:, (t + d) * K:(t + d + 1) * K],
                            start=(i == fidx[bank(t)]),
                            stop=(i == lidx[bank(t)]),
                        )
                    # drain wave PSUM -> bf16 cnt, 4-ish tiles per copy
                    for t in range(wa, wb, EB):
                        te = min(wb, t + EB)
                        nc.scalar.copy(
                            cntb[:, t * K:te * K],
                            pcn[:, (t - wa) * K:(te - wa) * K])
                    # w2 = cnt * se for the wave (DVE, bf16 2x)
                    nc.vector.tensor_mul(
                        w2b[:, wa * K:wb * K], cntb[:, wa * K:wb * K],
                        seb[:, wa * K:wb * K])
                    # VLAD for the wave; dummy absorbs the w2 DVE wait so
                    # the stream matmuls carry only the xvb DMA wait
                    dummy = pp.tile([128, K], f32, tag="dum", bufs=1)
                    nc.tensor.matmul(
                        dummy[:, 0:K],
                        lhsT=shb_sb[:, 0:128],
                        rhs=w2b[:, wa * K:(wa + 1) * K],
                        start=True, stop=True)
                    for t in range(wa, wb):
                        lt = w2b[:, t * K:(t + 1) * K]
                        base = (t - V0) * C1
                        nc.tensor.matmul(
                            pv0[:], lhsT=lt,
                            rhs=xvb_sb[:, base:base + C],
                            start=(t == CNW[0][0]), stop=(t == CNW[-1][1] - 1))
                        nc.tensor.matmul(
                            pv1[:, 0:1], lhsT=lt,
                            rhs=xvb_sb[:, base + C:base + C1],
                            start=(t == CNW[0][0]), stop=(t == CNW[-1][1] - 1))

                top2_chunk(0)
                for ci in range(1, len(TPB)):
                    top2_chunk(ci)
                    wave(ci - 1)
                wave(len(CNW) - 1)

                # ---- drain partials; host sums cores + normalizes
                nc.scalar.copy(vl_sb[:, 0:C], pv0[:])
                nc.scalar.copy(vl_sb[:, C:C1], pv1[:, 0:1])
                nc.sync.dma_start(y[:], vl_sb[:])
    _prune_waits(nc)
    return nc


def _prune_waits(nc):
    """Drop semaphore waits transitively implied by another wait on the same
    instruction (the walrus codegen allows one sync wait per instruction).
    Per-engine queues execute in-order, so a later instruction's completion
    implies every earlier same-engine instruction's waits held (including
    non-sem-updating ones like InstLdweights)."""
    insts = [ins for bb in nc.main_func.blocks for ins in bb.instructions]
    proc_events = {}
    waits_of = {}
    stream_of = {}           # id(ins) -> (engine_key, index)
    stream_cum = {}          # engine_key -> list of cumulative wait-unions
    for ins in insts:
        si = getattr(ins, "sync_info", None)
        if si is None:
            continue
        ow = list(si.on_wait or [])
        waits_of[id(ins)] = [(w.ant_name, w.wait_value) for w in ow]
        eng = str(getattr(ins, "engine", None))
        if eng and "Pool" not in eng:
            lst = stream_cum.setdefault(eng, [])
            cur = dict(lst[-1]) if lst else {}
            for (s2, v2) in waits_of[id(ins)]:
                if cur.get(s2, 0) < v2:
                    cur[s2] = v2
            stream_of[id(ins)] = (eng, len(lst))
            lst.append(cur)
        for u in (si.on_update or []):
            if getattr(u, "update_mode", None) not in ("sem-inc", "sem-add-imm"):
                continue
            lst = proc_events.setdefault(u.ant_name, [])
            prev = lst[-1][0] if lst else 0
            lst.append((prev + (u.update_value or 1), ins))

    import bisect

    def holds(sem, v):
        """Exact transitive closure of thresholds implied by sem >= v."""
        out = {}
        work = [(sem, v)]
        while work:
            s, t = work.pop()
            lst = proc_events.get(s)
            if not lst:
                continue
            ticks = [tk for tk, _ in lst]
            i = bisect.bisect_left(ticks, t)
            if i >= len(lst):
                continue
            implied = {}
            for j in range(i + 1):
                _, ins = lst[j]
                st = stream_of.get(id(ins))
                if st is not None:
                    for (s2, v2) in stream_cum[st[0]][st[1]].items():
                        if implied.get(s2, 0) < v2:
                            implied[s2] = v2
                else:
                    for (s2, v2) in waits_of.get(id(ins), []):
                        if implied.get(s2, 0) < v2:
                            implied[s2] = v2
            for s2, v2 in implied.items():
                if out.get(s2, 0) < v2:
                    out[s2] = v2
                    work.append((s2, v2))
        return out

    own_tick = {}
    for sem, lst in proc_events.items():
        for tick, ins in lst:
            own_tick[(id(ins), sem)] = tick

    pruned = 0
    for ins in insts:
        si = getattr(ins, "sync_info", None)
        if si is None or not si.on_wait or len(si.on_wait) < 2:
            continue
        ow = list(si.on_wait)
        kept = list(ow)
        for w in ow:
            if len(kept) == 1:
                break
            mine = own_tick.get((id(ins), w.ant_name))
            if mine is not None and w.wait_value <= mine - 1:
                kept.remove(w)
                pruned += 1
                continue
            others = [o for o in kept if o is not w]
            for o in others:
                h = holds(o.ant_name, o.wait_value)
                if h.get(w.ant_name, 0) >= w.wait_value:
                    kept.remove(w)
                    pruned += 1
                    break
        si.on_wait = kept
    return pruned


def _host_prep(x, conv_w, centroids):
    from concourse import mybir
    bf16np = mybir.dt.np(mybir.dt.bfloat16)
    fp8np = mybir.dt.np(mybir.dt.float8e4)

    x = np.ascontiguousarray(x, dtype=np.float32)
    L = H * W
    norm = np.sqrt((x.astype(np.float64) ** 2).sum(0))
    inv_norm = (1.0 / np.maximum(norm, 1e-12)).astype(np.float32)  # [H,W]
    xn = x * inv_norm[None]                                  # normalized x
    ii = np.arange(H, dtype=np.float32)
    mi = np.minimum(H - 1 - ii, ii)
    m = np.minimum(mi[:, None], mi[None, :]).astype(np.float32)
    m2 = m * m
    mask4 = m2 * m2                                          # [H,W]

    xpad8 = np.zeros((C, H + 2, W), fp8np)
    xpad8[:, 1:H + 1, :] = xn.astype(fp8np)
    xnb_pad = np.zeros(((H + 2) * W, C), bf16np)             # [Lpad, C] bf16
    xnb_pad[W:(H + 1) * W] = xn.reshape(C, L).T.astype(bf16np)
    sc_pad = np.zeros((H + 2) * W, np.float32)
    sc_pad[W:(H + 1) * W] = mask4.reshape(L)

    cwb = np.ascontiguousarray(
        conv_w.astype(np.float32).T.reshape(CT, 128, K)
        .transpose(1, 0, 2).reshape(128, CT * K)).astype(fp8np)

    # 5 banded 0/1 shift matrices: A_d[p', p] = 1 iff 128d + p' - p in D
    D = {di * W + dj for di in (-1, 0, 1) for dj in (-1, 0, 1)}
    shb = np.zeros((128, 5 * 128), bf16np)
    pp_, p_ = np.meshgrid(np.arange(128), np.arange(128), indexing="ij")
    for j, d in enumerate(range(-2, 3)):
        band = np.isin(128 * d + pp_ - p_, list(D))
        shb[:, j * 128:(j + 1) * 128] = band.astype(bf16np)

    V0, V1 = CNW[0][0], CNW[-1][1]
    NV = V1 - V0

    in_maps = []
    for core in range(M):
        r0 = core * RPC
        sl = slice(r0 * W, (r0 + RPC + 2) * W)               # slab, padded coords
        sc0c = sc_pad[sl].copy()
        sc0c[0:W] = 0.0                                      # halo rows give 0
        sc0c[(RPC + 1) * W:] = 0.0
        # xtb: [128, (t, ct, li)] = xn[ct*128+p, t*128+li] (fp8)
        xs = xpad8[:, r0:r0 + RPC + 2, :].reshape(C, Ls)
        xtb = np.ascontiguousarray(
            xs.reshape(CT, 128, NT, 128).transpose(1, 2, 0, 3)
            .reshape(128, NT * CT * 128))
        # xvb: [128, (t-V0, c)] = xnT[t*128+p, c], ones column; tiles V0..V1
        lo = r0 * W + V0 * 128                               # padded coords
        xv = np.ones((NV * 128, C1), bf16np)
        xv[:, 0:C] = xnb_pad[lo:lo + NV * 128]
        xvb = np.ascontiguousarray(
            xv.reshape(NV, 128, C1).transpose(1, 0, 2).reshape(128, NV * C1))
        in_maps.append({
            "xtb": xtb,
            "xvb": xvb,
            "cwb": cwb,
            "shb": shb,
            "cst": np.ascontiguousarray(sc0c.reshape(NT, 128).T,
                                        dtype=np.float32),
        })
    return in_maps


def _ensure_ntff_hook():
    """Install the axon NTFF profile hook if the image's antenv lacks it."""
    import types
    try:
        from antenv.axon_hooks import get_axon_ntff_profile_hook  # noqa: F401
        return
    except ImportError:
        pass
    if "/root/.axon_site" not in sys.path:
        sys.path.insert(0, "/root/.axon_site")
    from trn_agent_boot.trn_boot import _ntff_profile_via_ctypes
    hook = _ntff_profile_via_ctypes("/opt/axon/libaxon_pjrt.so")
    mod = types.ModuleType("antenv.axon_hooks")
    mod.get_axon_ntff_profile_hook = lambda: hook
    mod.set_axon_ntff_profile_hook = lambda h: None
    import antenv
    antenv.axon_hooks = mod
    sys.modules["antenv.axon_hooks"] = mod


def _install_neff_cache():
    """Cache compiled NEFFs across processes, keyed by BIR content hash."""
    import hashlib
    import shutil
    import concourse.bass2jax as b2j

    orig = b2j.compile_bir_kernel
    if getattr(orig, "_neff_cached", False):
        return

    def cached(bir_json, tmpdir, neff_name="file.neff"):
        h = hashlib.sha256(
            bir_json if isinstance(bir_json, bytes) else bir_json.encode()
        ).hexdigest()[:24]
        cdir = "/tmp/neff_cache"
        os.makedirs(cdir, exist_ok=True)
        cpath = os.path.join(cdir, h + ".neff")
        if os.path.exists(cpath):
            dst = os.path.join(tmpdir, neff_name)
            os.makedirs(tmpdir, exist_ok=True)
            shutil.copy(cpath, dst)
            return dst
        out = orig(bir_json, tmpdir, neff_name=neff_name)
        shutil.copy(out, cpath)
        return out

    cached._neff_cached = True
    b2j.compile_bir_kernel = cached


def kernel(x, conv_w, centroids):
    import concourse.bass_utils as bu
    from concourse.bass_utils import run_bass_kernel_spmd
    _install_neff_cache()
    if TRACE:
        _ensure_ntff_hook()
        bu.upload_artifacts = lambda tmpdir: "local://" + tmpdir

    if "nc" not in _CACHE:
        _CACHE["nc"] = _build_nc()
    nc = _CACHE["nc"]
    in_maps = _host_prep(np.asarray(x), np.asarray(conv_w), np.asarray(centroids))
    res = run_bass_kernel_spmd(nc, in_maps, list(range(M)), trace=TRACE)
    _CACHE["last"] = res
    red = np.zeros((K, C1), np.float32)
    for r in res.results:
        red += np.asarray(r["y"], dtype=np.float32)
    vlad = red[:, :C] - red[:, C:C1] * np.asarray(centroids, np.float32)
    vlad /= np.maximum(np.sqrt((vlad ** 2).sum(1))[:, None], 1e-12)
    v = vlad.reshape(1, K * C)
    v /= np.maximum(np.sqrt((v ** 2).sum()), 1e-12)
    return v.astype(np.float32)


# revision 24
# speedup vs baseline: 3.2733x; 1.0543x over previous
"""NetVLAD (vq_codebook) Trainium2 Bass kernel, 8-way spatially sharded. v3.

Math (same identity as v1/v2):
  xn = x / ||x||_C per location; logits = conv_w @ xn; soft = softmax_K
  fold(unfold(soft) * top2keep) == soft * cnt, cnt = 3x3 box-sum of the
  per-cluster top-2 indicator (border wrap artifacts killed by the
  (min-dist-to-border)^4 mask). vlad = sa2 @ xn.T - rowsum(sa2) * centroids.

v3 strategy (all compute in [L-partition, K-free] layout, no transposes):
  - x is L2-normalized on the host and streamed quantized in BOTH layouts:
    fp8e4m3 [C,L]-tiled for the logits lhsT, bf16 [L,C+1] for the VLAD
    moving operand (+ a ones column that yields rowsum(sa2) for free).
  - logits computed directly into [l-tile, K] PSUM banks, 4 tiles/bank;
    one batched exp per bank (no scale, no accumulator read).
  - softmax sum + top-2 keep on DVE in bf16 chunks; cnt = 3x3 box-sum on
    the PE as 5 banded 128x128 0/1 matmuls per tile, d-outer waves.
  - per-core [K, C+1] partials reduced on host (0.03% of FLOPs).
  - xvb stream chunks are gated behind scalar-engine progress via
    WAR deps (a scalar touch reads the chunk region before the DMA
    writes it) so the Tile scheduler cannot front-run the x load.

Sharding: H=192 rows split 8 ways (24 rows/core + 1 halo row each side).
"""
import os
import sys

sys.path.insert(0, "/opt/trn_rl_repo")
os.environ.setdefault("MYCRO_LOCAL_CACHE", "1")

import numpy as np

C, H, W, K = 512, 192, 192, 64
M = 8                       # cores
RPC = H // M                # 24 rows per core
Ls = (RPC + 2) * W          # 4992 slab locations (incl. 1 halo row each side)
NT = Ls // 128              # 39 l-tiles
CT = C // 128               # 4 c-tiles
C1 = C + 1                  # x columns + ones column
XTG = [6, 9, 11, 13]        # xtb DMA chunk sizes (tiles), staggered completion
TPB = [(0, 10), (10, 20), (20, 30), (30, 39)]   # top2 chunks
CNW = [(1, 8), (8, 18), (18, 28), (28, 38)]     # cnt waves == vlad groups
EB = 4                      # exp batch: tiles per PSUM bank
GATES = {1: 0, 3: 1, 5: 2, 7: 3}  # exp-batch index -> xvb chunk after it

TRACE = False               # set by test.py for profiling runs
_CACHE = {}


def _build_nc():
    import concourse.bass as bass
    import concourse.tile as tile
    from concourse import mybir

    f32 = mybir.dt.float32
    bf16 = mybir.dt.bfloat16
    fp8 = mybir.dt.float8e4
    AF = mybir.ActivationFunctionType
    OP = mybir.AluOpType
    AX = mybir.AxisListType

    NV = CNW[-1][1] - CNW[0][0]        # 37 tiles carried in xvb (skip 0, 38)
    V0 = CNW[0][0]

    nc = bass.Bass()
    xtb = nc.dram_tensor("xtb", [128, NT * CT * 128], fp8, kind="ExternalInput")
    xvb = nc.dram_tensor("xvb", [128, NV * C1], bf16, kind="ExternalInput")
    cwb = nc.dram_tensor("cwb", [128, CT * K], fp8, kind="ExternalInput")
    shb = nc.dram_tensor("shb", [128, 5 * 128], bf16, kind="ExternalInput")
    cst = nc.dram_tensor("cst", [128, NT], f32, kind="ExternalInput")
    y = nc.dram_tensor("y", [K, C1], f32, kind="ExternalOutput")

    xtg = np.cumsum([0] + XTG)

    with tile.TileContext(nc) as tc:
        with tc.tile_pool(name="big", bufs=1) as big:
            xtb_sb = big.tile([128, NT * CT * 128], fp8, tag="xtb")
            xvb_sb = big.tile([128, NV * C1], bf16, tag="xvb")
            cwb_sb = big.tile([128, CT * K], fp8, tag="cwb")
            shb_sb = big.tile([128, 5 * 128], bf16, tag="shb")
            sc0 = big.tile([128, NT], f32, tag="sc0")
            expb = big.tile([128, NT * K], bf16, tag="expb")
            keep = big.tile([128, NT * K], bf16, tag="keep")
            seb = big.tile([128, NT * K], bf16, tag="seb")
            w2b = big.tile([128, NT * K], bf16, tag="w2b")
            sume = big.tile([128, NT], f32, tag="sume")
            isum = big.tile([128, NT], f32, tag="isum")
            scc = big.tile([128, NT], f32, tag="scc")
            m8 = big.tile([128, NT * 8], bf16, tag="m8")
            vl_sb = big.tile([K, C1], f32, tag="vl")
            scr = big.tile([128, 4], f32, tag="scr")

            # cwb (needed by the first warmup + phase 1) goes first on the
            # sync DGE; the other constants ride the scalar HWDGE so the
            # xtb stream descriptors start generating immediately after
            nc.sync.dma_start(cwb_sb[:], cwb[:])
            nc.scalar.dma_start(shb_sb[:], shb[:])
            nc.scalar.dma_start(sc0[:], cst[:])
            for g in range(len(XTG)):
                a, b = int(xtg[g]) * CT * 128, int(xtg[g + 1]) * CT * 128
                nc.sync.dma_start(xtb_sb[:, a:b], xtb[:, a:b])

            # single-wait touch absorbing the cst DMA for later DVE/scalar use
            nc.scalar.copy(scr[:, 0:1], sc0[:, 0:1])
            # init the xvb gate columns so the gate touches read defined data
            for (va, _vb) in CNW:
                a = (va - V0) * C1
                nc.vector.memset(xvb_sb[:, a:a + 1], 0.0)

            with tc.tile_pool(name="pp", bufs=1, space="PSUM") as pp:
                # all PSUM tiles are whole-bank (2048B) multiples so every
                # allocation is bank-aligned and matmuls never cross banks:
                # pcn 2 + pv0 1 + pv1 1 + plg 3 + dum 1 = 8 banks
                pcn = pp.tile([128, 2 * C], f32, tag="pcn", bufs=1)
                pv0 = pp.tile([K, C], f32, tag="pv0", bufs=1)
                pv1 = pp.tile([K, C], f32, tag="pv1", bufs=1)
                plg0 = pp.tile([128, C], f32, tag="plg", bufs=3,
                               name="plg0")  # reserve bank-aligned slots

                # warm-up burst: absorbs cwb/shb DMA waits, spins up the HAM
                dummy = pp.tile([128, K], f32, tag="dum", bufs=1)
                nc.tensor.matmul(dummy[0:64, 0:64], lhsT=cwb_sb[:, 0:64],
                                 rhs=cwb_sb[:, 0:64], start=True, stop=True)
                for _ in range(6):
                    dummy = pp.tile([128, K], f32, tag="dum", bufs=1)
                    nc.tensor.matmul(dummy[:, 0:64], lhsT=shb_sb[:, 0:128],
                                     rhs=shb_sb[:, 0:64], start=True, stop=True)

                # ---- phase 1: logits into [l-tile, K] PSUM, 4 tiles/bank;
                # batched exp per bank (x pre-normalized, so no scale)
                nb = (NT + EB - 1) // EB
                gate_cols = {}
                plogs = {}
                cg = 0                      # xtb chunk cursor

                DR = mybir.MatmulPerfMode.DoubleRow
                for bi in range(nb):
                    t0, t1 = bi * EB, min(NT, (bi + 1) * EB)
                    plg = pp.tile([128, C], f32, tag="plg", bufs=3)
                    plogs[bi] = plg
                    for t in range(t0, t1):
                        if cg < len(XTG) and t == int(xtg[cg]):
                            # dummy matmul absorbs this xtb chunk's DMA wait
                            dummy = pp.tile([128, K], f32, tag="dum", bufs=1)
                            base = t * CT * 128
                            nc.tensor.matmul(
                                dummy[:, 0:64],
                                lhsT=xtb_sb[:, base:base + 128],
                                rhs=xtb_sb[:, base:base + 64],
                                start=True, stop=True)
                            cg += 1
                        # fp8 DoubleRow: 256-deep contraction per matmul,
                        # c-halves stacked in the AP middle dim
                        for h in range(2):
                            lv = xtb_sb[:, (t * CT + 2 * h) * 128:
                                        (t * CT + 2 * h + 2) * 128]
                            rv = cwb_sb[:, 2 * h * K:(2 * h + 2) * K]
                            nc.tensor.matmul(
                                plg[:, (t - t0) * K:(t - t0 + 1) * K],
                                lhsT=lv.rearrange("p (two f) -> p two f",
                                                  two=2),
                                rhs=rv.rearrange("p (two f) -> p two f",
                                                 two=2),
                                start=(h == 0), stop=(h == 1),
                                perf_mode=DR,
                            )
                    nc.scalar.activation(
                        expb[:, t0 * K:t1 * K], plg[:, 0:(t1 - t0) * K],
                        AF.Exp)
                    if bi in GATES:
                        # gate: touch-read the xvb chunk region, then issue
                        # its DMA; the WAR dep paces the stream off scalar
                        # progress (the Tile scheduler keeps DMA after touch)
                        gi = GATES[bi]
                        va, vb = CNW[gi]
                        a = (va - V0) * C1
                        b = (vb - V0) * C1
                        nc.scalar.copy(scr[:, 2:3], xvb_sb[:, a:a + 1])
                        nc.sync.dma_start(xvb_sb[:, a:b], xvb[:, a:b])

                # ---- phases 2-4, software-pipelined: top2 chunk ci, then
                # wave ci-1's [cnt matmuls -> scalar drain -> w2 -> VLAD]
                # (wave i only needs keep from chunk i, so trailing by one
                # chunk keeps every engine busy without stalls)

                def top2_chunk(ci):
                    ta, tb = TPB[ci]
                    n = tb - ta
                    e3 = expb[:, ta * K:tb * K].rearrange(
                        "p (t k) -> p t k", k=K)
                    k3 = keep[:, ta * K:tb * K].rearrange(
                        "p (t k) -> p t k", k=K)
                    s3 = seb[:, ta * K:tb * K].rearrange(
                        "p (t k) -> p t k", k=K)
                    # per-tile top-8 (Max8); m2 = column 1
                    for t in range(ta, tb):
                        nc.vector.max(m8[:, t * 8:(t + 1) * 8],
                                      expb[:, t * K:(t + 1) * K])
                    m83 = m8[:, ta * 8:tb * 8].rearrange(
                        "p (t e) -> p t e", e=8)
                    m2c = m83[:, :, 1:2].broadcast_to([128, n, K])
                    scc_c = scc[:, ta:tb][:, :, None].broadcast_to([128, n, K])
                    nc.vector.tensor_tensor(k3, e3, m2c, op=OP.is_ge)
                    nc.vector.tensor_reduce(
                        sume[:, ta:tb], e3, axis=AX.X, op=OP.add)
                    nc.vector.reciprocal(isum[:, ta:tb], sume[:, ta:tb])
                    nc.vector.tensor_mul(
                        scc[:, ta:tb], sc0[:, ta:tb], isum[:, ta:tb])
                    nc.vector.tensor_tensor(s3, e3, scc_c, op=OP.mult)

                def wave(wi):
                    wa, wb = CNW[wi]
                    # dummy absorbs this wave's keep (DVE) wait
                    dummy = pp.tile([128, K], f32, tag="dum", bufs=1)
                    nc.tensor.matmul(
                        dummy[:, 0:K],
                        lhsT=shb_sb[:, 0:128],
                        rhs=keep[:, wa * K:(wa + 1) * K],
                        start=True, stop=True)
                    # d-outer banded matmuls: one LDWEIGHTS per shift matrix.
                    # start/stop once per 2KB PSUM zero region (8 tiles/bank):
                    # start=True lazily zeroes the bank; first-touch writes of
                    # other tiles store, later touches accumulate.
                    pairs = [(d, t) for d in range(-2, 3)
                             for t in range(wa, wb) if 0 <= t + d < NT]
                    bank = lambda t: (t - wa) // 8
                    fidx = {}
                    lidx = {}
                    for i, (d, t) in enumerate(pairs):
                        fidx.setdefault(bank(t), i)
                        lidx[bank(t)] = i
                    for i, (d, t) in enumerate(pairs):
                        nc.tensor.matmul(
                            pcn[:, (t - wa) * K:(t - wa + 1) * K],
                            lhsT=shb_sb[:, (d + 2) * 128:(d + 3) * 128],
                            rhs=keep[:, (t + d) * K:(t + d + 1) * K],
                            start=(i == fidx[bank(t)]),
                            stop=(i == lidx[bank(t)]),
                        )
                    # w2 = cnt * se straight from PSUM (one DVE op per wave;
                    # no scalar drain hop in the wave handoff)
                    nc.vector.tensor_mul(
                        w2b[:, wa * K:wb * K],
                        pcn[:, 0:(wb - wa) * K],
                        seb[:, wa * K:wb * K])
                    # VLAD for the wave; dummy absorbs the w2 DVE wait so
                    # the stream matmuls carry only the xvb DMA wait
                    dummy = pp.tile([128, K], f32, tag="dum", bufs=1)
                    nc.tensor.matmul(
                        dummy[:, 0:K],
                        lhsT=shb_sb[:, 0:128],
                        rhs=w2b[:, wa * K:(wa + 1) * K],
                        start=True, stop=True)
                    for t in range(wa, wb):
                        lt = w2b[:, t * K:(t + 1) * K]
                        base = (t - V0) * C1
                        nc.tensor.matmul(
                            pv0[:], lhsT=lt,
                            rhs=xvb_sb[:, base:base + C],
                            start=(t == CNW[0][0]), stop=(t == CNW[-1][1] - 1))
                        nc.tensor.matmul(
                            pv1[:, 0:1], lhsT=lt,
                            rhs=xvb_sb[:, base + C:base + C1],
                            start=(t == CNW[0][0]), stop=(t == CNW[-1][1] - 1))

                top2_chunk(0)
                for ci in range(1, len(TPB)):
                    top2_chunk(ci)
                    wave(ci - 1)
                wave(len(CNW) - 1)

                # ---- drain partials; host sums cores + normalizes
                nc.scalar.copy(vl_sb[:, 0:C], pv0[:])
                nc.scalar.copy(vl_sb[:, C:C1], pv1[:, 0:1])
                nc.sync.dma_start(y[:], vl_sb[:])
    _prune_waits(nc)
    return nc


def _prune_waits(nc):
    """Drop semaphore waits transitively implied by another wait on the same
    instruction (the walrus codegen allows one sync wait per instruction).
    Per-engine queues execute in-order, so a later instruction's completion
    implies every earlier same-engine instruction's waits held (including
    non-sem-updating ones like InstLdweights)."""
    insts = [ins for bb in nc.main_func.blocks for ins in bb.instructions]
    proc_events = {}
    waits_of = {}
    stream_of = {}           # id(ins) -> (engine_key, index)
    stream_cum = {}          # engine_key -> list of cumulative wait-unions
    for ins in insts:
        si = getattr(ins, "sync_info", None)
        if si is None:
            continue
        ow = list(si.on_wait or [])
        waits_of[id(ins)] = [(w.ant_name, w.wait_value) for w in ow]
        eng = str(getattr(ins, "engine", None))
        if eng and "Pool" not in eng:
            lst = stream_cum.setdefault(eng, [])
            cur = dict(lst[-1]) if lst else {}
            for (s2, v2) in waits_of[id(ins)]:
                if cur.get(s2, 0) < v2:
                    cur[s2] = v2
            stream_of[id(ins)] = (eng, len(lst))
            lst.append(cur)
        for u in (si.on_update or []):
            if getattr(u, "update_mode", None) not in ("sem-inc", "sem-add-imm"):
                continue
            lst = proc_events.setdefault(u.ant_name, [])
            prev = lst[-1][0] if lst else 0
            lst.append((prev + (u.update_value or 1), ins))

    import bisect

    def holds(sem, v):
        """Exact transitive closure of thresholds implied by sem >= v."""
        out = {}
        work = [(sem, v)]
        while work:
            s, t = work.pop()
            lst = proc_events.get(s)
            if not lst:
                continue
            ticks = [tk for tk, _ in lst]
            i = bisect.bisect_left(ticks, t)
            if i >= len(lst):
                continue
            implied = {}
            for j in range(i + 1):
                _, ins = lst[j]
                st = stream_of.get(id(ins))
                if st is not None:
                    for (s2, v2) in stream_cum[st[0]][st[1]].items():
                        if implied.get(s2, 0) < v2:
                            implied[s2] = v2
                else:
                    for (s2, v2) in waits_of.get(id(ins), []):
                        if implied.get(s2, 0) < v2:
                            implied[s2] = v2
            for s2, v2 in implied.items():
                if out.get(s2, 0) < v2:
                    out[s2] = v2
                    work.append((s2, v2))
        return out

    own_tick = {}
    for sem, lst in proc_events.items():
        for tick, ins in lst:
            own_tick[(id(ins), sem)] = tick

    pruned = 0
    for ins in insts:
        si = getattr(ins, "sync_info", None)
        if si is None or not si.on_wait or len(si.on_wait) < 2:
            continue
        ow = list(si.on_wait)
        kept = list(ow)
        for w in ow:
            if len(kept) == 1:
                break
            mine = own_tick.get((id(ins), w.ant_name))
            if mine is not None and w.wait_value <= mine - 1:
                kept.remove(w)
                pruned += 1
                continue
            others = [o for o in kept if o is not w]
            for o in others:
                h = holds(o.ant_name, o.wait_value)
                if h.get(w.ant_name, 0) >= w.wait_value:
                    kept.remove(w)
                    pruned += 1
                    break
        si.on_wait = kept
    return pruned


def _host_prep(x, conv_w, centroids):
    from concourse import mybir
    bf16np = mybir.dt.np(mybir.dt.bfloat16)
    fp8np = mybir.dt.np(mybir.dt.float8e4)

    x = np.ascontiguousarray(x, dtype=np.float32)
    L = H * W
    norm = np.sqrt((x.astype(np.float64) ** 2).sum(0))
    inv_norm = (1.0 / np.maximum(norm, 1e-12)).astype(np.float32)  # [H,W]
    xn = x * inv_norm[None]                                  # normalized x
    ii = np.arange(H, dtype=np.float32)
    mi = np.minimum(H - 1 - ii, ii)
    m = np.minimum(mi[:, None], mi[None, :]).astype(np.float32)
    m2 = m * m
    mask4 = m2 * m2                                          # [H,W]

    xpad8 = np.zeros((C, H + 2, W), fp8np)
    xpad8[:, 1:H + 1, :] = xn.astype(fp8np)
    xnb_pad = np.zeros(((H + 2) * W, C), bf16np)             # [Lpad, C] bf16
    xnb_pad[W:(H + 1) * W] = xn.reshape(C, L).T.astype(bf16np)
    sc_pad = np.zeros((H + 2) * W, np.float32)
    sc_pad[W:(H + 1) * W] = mask4.reshape(L)

    cwb = np.ascontiguousarray(
        conv_w.astype(np.float32).T.reshape(CT, 128, K)
        .transpose(1, 0, 2).reshape(128, CT * K)).astype(fp8np)

    # 5 banded 0/1 shift matrices: A_d[p', p] = 1 iff 128d + p' - p in D
    D = {di * W + dj for di in (-1, 0, 1) for dj in (-1, 0, 1)}
    shb = np.zeros((128, 5 * 128), bf16np)
    pp_, p_ = np.meshgrid(np.arange(128), np.arange(128), indexing="ij")
    for j, d in enumerate(range(-2, 3)):
        band = np.isin(128 * d + pp_ - p_, list(D))
        shb[:, j * 128:(j + 1) * 128] = band.astype(bf16np)

    V0, V1 = CNW[0][0], CNW[-1][1]
    NV = V1 - V0

    in_maps = []
    for core in range(M):
        r0 = core * RPC
        sl = slice(r0 * W, (r0 + RPC + 2) * W)               # slab, padded coords
        sc0c = sc_pad[sl].copy()
        sc0c[0:W] = 0.0                                      # halo rows give 0
        sc0c[(RPC + 1) * W:] = 0.0
        # xtb: [128, (t, ct, li)] = xn[ct*128+p, t*128+li] (fp8)
        xs = xpad8[:, r0:r0 + RPC + 2, :].reshape(C, Ls)
        xtb = np.ascontiguousarray(
            xs.reshape(CT, 128, NT, 128).transpose(1, 2, 0, 3)
            .reshape(128, NT * CT * 128))
        # xvb: [128, (t-V0, c)] = xnT[t*128+p, c], ones column; tiles V0..V1
        lo = r0 * W + V0 * 128                               # padded coords
        xv = np.ones((NV * 128, C1), bf16np)
        xv[:, 0:C] = xnb_pad[lo:lo + NV * 128]
        xvb = np.ascontiguousarray(
            xv.reshape(NV, 128, C1).transpose(1, 0, 2).reshape(128, NV * C1))
        in_maps.append({
            "xtb": xtb,
            "xvb": xvb,
            "cwb": cwb,
            "shb": shb,
            "cst": np.ascontiguousarray(sc0c.reshape(NT, 128).T,
                                        dtype=np.float32),
        })
    return in_maps


def _ensure_ntff_hook():
    """Install the axon NTFF profile hook if the image's antenv lacks it."""
    import types
    try:
        from antenv.axon_hooks import get_axon_ntff_profile_hook  # noqa: F401
        return
    except ImportError:
        pass
    if "/root/.axon_site" not in sys.path:
        sys.path.insert(0, "/root/.axon_site")
    from trn_agent_boot.trn_boot import _ntff_profile_via_ctypes
    hook = _ntff_profile_via_ctypes("/opt/axon/libaxon_pjrt.so")
    mod = types.ModuleType("antenv.axon_hooks")
    mod.get_axon_ntff_profile_hook = lambda: hook
    mod.set_axon_ntff_profile_hook = lambda h: None
    import antenv
    antenv.axon_hooks = mod
    sys.modules["antenv.axon_hooks"] = mod


def _install_neff_cache():
    """Cache compiled NEFFs across processes, keyed by BIR content hash."""
    import hashlib
    import shutil
    import concourse.bass2jax as b2j

    orig = b2j.compile_bir_kernel
    if getattr(orig, "_neff_cached", False):
        return

    def cached(bir_json, tmpdir, neff_name="file.neff"):
        h = hashlib.sha256(
            bir_json if isinstance(bir_json, bytes) else bir_json.encode()
        ).hexdigest()[:24]
        cdir = "/tmp/neff_cache"
        os.makedirs(cdir, exist_ok=True)
        cpath = os.path.join(cdir, h + ".neff")
        if os.path.exists(cpath):
            dst = os.path.join(tmpdir, neff_name)
            os.makedirs(tmpdir, exist_ok=True)
            shutil.copy(cpath, dst)
            return dst
        out = orig(bir_json, tmpdir, neff_name=neff_name)
        shutil.copy(out, cpath)
        return out

    cached._neff_cached = True
    b2j.compile_bir_kernel = cached


def kernel(x, conv_w, centroids):
    import concourse.bass_utils as bu
    from concourse.bass_utils import run_bass_kernel_spmd
    _install_neff_cache()
    if TRACE:
        _ensure_ntff_hook()
        bu.upload_artifacts = lambda tmpdir: "local://" + tmpdir

    if "nc" not in _CACHE:
        _CACHE["nc"] = _build_nc()
    nc = _CACHE["nc"]
    in_maps = _host_prep(np.asarray(x), np.asarray(conv_w), np.asarray(centroids))
    res = run_bass_kernel_spmd(nc, in_maps, list(range(M)), trace=TRACE)
    _CACHE["last"] = res
    red = np.zeros((K, C1), np.float32)
    for r in res.results:
        red += np.asarray(r["y"], dtype=np.float32)
    vlad = red[:, :C] - red[:, C:C1] * np.asarray(centroids, np.float32)
    vlad /= np.maximum(np.sqrt((vlad ** 2).sum(1))[:, None], 1e-12)
    v = vlad.reshape(1, K * C)
    v /= np.maximum(np.sqrt((v ** 2).sum()), 1e-12)
    return v.astype(np.float32)
